# revision 1
# baseline (speedup 1.0000x reference)
"""EnVAE sampling kernel for 8x TRN2 NeuronCores.

Math (per group g, batch element b):
  Xg = X[:, g::8]                                     # (b, 128)
  h  = relu(Xg @ W1[g] + b1[g])                        # (b, 128)
  out= h @ W2[g] + b2[g]; means=out[:, :64]; lv=out[:, 64:]
  z  = means[b, idx] + eps * exp(0.5 * lv[b, idx])

Device computes (batch-sharded 8 ways, fp16 matmuls):
  zM[g,b] = (W2m[g]^T h)[idx[g,b], b]        (via onehot Hadamard + reduce-matmul)
  zX[g,b] = exp(0.5*L + 0.5*b2v[g])[idx[g,b], b]
Host finishes: z = zM + b2m[g, idx] + eps * zX
"""

import numpy as np
import ml_dtypes

import concourse.bass as bass
import concourse.bacc as bacc
import concourse.mybir as mybir
from concourse import tile
from concourse import bass_utils

OBS = 1024
LAT = 64
G = 8
GS = 128
HID = 128
BATCH = 65536
NCORES = 8
BPC = BATCH // NCORES        # 8192 batch rows per core
SC = 512                     # batch rows per superchunk
NPAIR = G // 2
BF16 = mybir.dt.float16  # fp16: same PE rate as bf16, 8x mantissa
F32 = mybir.dt.float32

# group n takes columns n, n+8, ... (round-robin)
GROUP_IDX = np.stack([np.arange(n, OBS, G) for n in range(G)])  # (g, gs)


def build_program(nsc: int, num_devices: int = NCORES):
    """Build the per-core bass program for nsc superchunks of SC batch rows."""
    B = nsc * SC
    nc = bacc.Bacc("TRN2", target_bir_lowering=False, debug=False,
                   num_devices=num_devices)

    QUAD = 4 if nsc % 4 == 0 else 1
    nquad = nsc // QUAD
    QW = QUAD * SC
    # DRAM inputs (per-core shard)
    # xt: quad-block-major [nquad, G, QW, GS] fp16
    xt = nc.dram_tensor("xt", [nquad, G, QW, GS], BF16, kind="ExternalInput").ap()
    # onehot, transposed per pair: [nquad, NPAIR, 128, QW] int8
    #   partitions 0:64   = onehot[g=2*pair]   (latent on partition)
    #   partitions 64:128 = onehot[g=2*pair+1]
    oh = nc.dram_tensor("oh", [nquad, NPAIR, 128, QW], mybir.dt.int8,
                        kind="ExternalInput").ap()
    w1 = nc.dram_tensor("w1", [G, GS, HID], BF16, kind="ExternalInput").ap()
    # w2 packed per pair: [NPAIR, 2(tensor: m/v), GS, 2(group), LAT] bf16
    w2m = nc.dram_tensor("w2m", [G, GS, LAT], BF16, kind="ExternalInput").ap()
    w2v = nc.dram_tensor("w2v", [G, GS, LAT], BF16, kind="ExternalInput").ap()
    b1 = nc.dram_tensor("b1", [G, GS], F32, kind="ExternalInput").ap()
    # hb2v[pair] = per-partition bias col for exp: [NPAIR, 128] f32
    hb2v = nc.dram_tensor("hb2v", [NPAIR, 128], F32, kind="ExternalInput").ap()
    # selector for the reduce matmul: [2, 128, 4] bf16
    sel = nc.dram_tensor("sel", [2, 128, 4], BF16, kind="ExternalInput").ap()
    # output: [128, nsc*NPAIR*16] f32; col = ((sc*NPAIR + pair)*4 + c)*4 + q
    zout = nc.dram_tensor("z", [128, nsc * NPAIR * 16], F32,
                          kind="ExternalOutput").ap()

    from contextlib import ExitStack
    with tile.TileContext(nc) as tc, ExitStack() as st:
        # --- resident constants ---
        cp = st.enter_context(tc.tile_pool(name="const", bufs=1))
        if True:
            w1_sb = cp.tile([GS, G, HID], BF16, tag="w1")
            nc.sync.dma_start(w1_sb[:], w1.rearrange("g k m -> k g m"))
            w2m_sb = cp.tile([GS, G, LAT], BF16, tag="w2m")
            nc.sync.dma_start(w2m_sb[:], w2m.rearrange("g k m -> k g m"))
            w2v_sb = cp.tile([GS, G, LAT], BF16, tag="w2v")
            nc.sync.dma_start(w2v_sb[:], w2v.rearrange("g k m -> k g m"))
            b1_sb = cp.tile([GS, G], F32, tag="b1")
            nc.sync.dma_start(b1_sb[:], b1.rearrange("g k -> k g"))
            hb2v_sb = cp.tile([128, NPAIR], F32, tag="hb2v")
            nc.sync.dma_start(hb2v_sb[:], hb2v.rearrange("p k -> k p"))
            sel_sb = cp.tile([128, 2, 4], BF16, tag="sel")
            nc.sync.dma_start(sel_sb[:], sel.rearrange("t k f -> k t f"))

            # persistent z staging + z psum banks
            zpool = st.enter_context(tc.tile_pool(name="zp", bufs=1, space="PSUM"))
            xpool = st.enter_context(tc.tile_pool(name="xt", bufs=16))
            ohpool = st.enter_context(tc.tile_pool(name="ohp", bufs=8))
            hpsum = st.enter_context(tc.tile_pool(name="hps", bufs=3, space="PSUM"))
            hpool = st.enter_context(tc.tile_pool(name="hsb", bufs=8))
            mvpsum = st.enter_context(tc.tile_pool(name="mvps", bufs=2, space="PSUM"))
            ppool = st.enter_context(tc.tile_pool(name="prod", bufs=8))
            zsbp = st.enter_context(tc.tile_pool(name="zsb", bufs=1))

            if True:
                ZCOLS = 16  # cols per (pair, sc) in the z psum tile: 4 chunks x 4 q
                # one z psum tile per 32 (pair,sc) instances (512 cols each)
                nzt = (nsc * NPAIR + 31) // 32
                ztiles = [zpool.tile([128, 512], F32, name=f"zt{i}", tag="z")
                          for i in range(nzt)]
                zsb = zsbp.tile([128, nsc * NPAIR * 16], F32, tag="zstage")

                pending = []
                stage2 = []
                drained = set()

                def _emit_stage2(item):
                    inst, bM, bX, oht_, pr = item
                    prodM = ppool.tile([128, SC], BF16, name="prodM",
                                       tag="prodM")
                    nc.vector.tensor_tensor(prodM[:], bM[:], oht_,
                                            mybir.AluOpType.mult)
                    xsb = ppool.tile([128, SC], BF16, name="xsb", tag="xsb")
                    nc.scalar.activation(
                        xsb[:], bX[:],
                        mybir.ActivationFunctionType.Exp,
                        bias=hb2v_sb[:, pr:pr + 1], scale=0.5)
                    prodX = ppool.tile([128, SC], BF16, name="prodX",
                                       tag="prodX")
                    nc.gpsimd.tensor_tensor(prodX[:], xsb[:], oht_,
                                            mybir.AluOpType.mult)
                    pending.append((inst, prodM, prodX))

                def _drain(done_tile_idx):
                    # after the last zred of a z tile, copy it out so the
                    # single psum slot can recycle
                    if done_tile_idx is not None:
                        i = done_tile_idx
                        w = min(512, nsc * NPAIR * 16 - i * 512)
                        nc.scalar.copy(zsb[:, i * 512:i * 512 + w],
                                       ztiles[i][:, :w])
                        drained.add(i)

                def _emit_zred(item):
                    inst, pM, pX = item
                    zt = ztiles[inst // 32]
                    zoff = (inst % 32) * ZCOLS
                    for c in range(4):
                        zslice = zt[:, zoff + 4 * c: zoff + 4 * c + 4]
                        nc.tensor.matmul(
                            zslice, pM[:, 128 * c:128 * c + 128],
                            sel_sb[:, 0], start=True, stop=False,
                            skip_group_check=True)
                        nc.tensor.matmul(
                            zslice, pX[:, 128 * c:128 * c + 128],
                            sel_sb[:, 1], start=False, stop=True,
                            skip_group_check=True)

                for quad in range(nquad):
                    # --- bulk loads: XgT for all 8 groups, oh for all pairs
                    xg = [xpool.tile([GS, QW], BF16, name=f"xg{g}", tag="xg")
                          for g in range(G)]
                    for g in range(G):
                        nc.sync.dma_start(xg[g][:], xt[quad, g], transpose=True)
                    ohq = [ohpool.tile([128, QW], mybir.dt.int8,
                                        name=f"oh{p}", tag="oh")
                           for p in range(NPAIR)]
                    for p in range(NPAIR):
                        nc.sync.dma_start(ohq[p][:], oh[quad, p])

                    for scq in range(QUAD):
                        sc = quad * QUAD + scq
                        so = scq * SC
                        for pair in range(NPAIR):
                            g0, g1 = 2 * pair, 2 * pair + 1
                            oht = ohq[pair][:, so:so + SC]

                            # --- mm1 + relu per group (relu alternates ACT/DVE)
                            hsb = [hpool.tile([HID, SC], BF16, name=f"hsb{_i}",
                                              tag="h") for _i in range(2)]
                            for i, g in enumerate((g0, g1)):
                                hp = hpsum.tile([HID, SC], F32, tag="hpsum")
                                nc.tensor.matmul(hp[:], w1_sb[:, g],
                                                 xg[g][:, so:so + SC],
                                                 start=True, stop=True)
                                if i == 0:
                                    # g0 relu on ACT, g1 on DVE: they run
                                    # concurrently, unblocking mm2 sooner
                                    nc.scalar.activation(
                                        hsb[i][:], hp[:],
                                        mybir.ActivationFunctionType.Relu,
                                        bias=b1_sb[:, g:g + 1], scale=1.0)
                                else:
                                    nc.vector.tensor_scalar(
                                        hsb[i][:], hp[:],
                                        b1_sb[:, g:g + 1], 0.0,
                                        mybir.AluOpType.add,
                                        mybir.AluOpType.max)

                            # --- mm2: col-packed pairs (means first) ---
                            bankM = mvpsum.tile([128, SC], F32, tag="bankM")
                            bankX = mvpsum.tile([128, SC], F32, tag="bankX")
                            for i, g in enumerate((g0, g1)):
                                nc.tensor.matmul(bankM[64 * i:64 * i + 64, :],
                                                 w2m_sb[:, g], hsb[i][:],
                                                 start=True, stop=True,
                                                 tile_position=(0, 64 * i))
                            for i, g in enumerate((g0, g1)):
                                nc.tensor.matmul(bankX[64 * i:64 * i + 64, :],
                                                 w2v_sb[:, g], hsb[i][:],
                                                 start=True, stop=True,
                                                 tile_position=(0, 64 * i))

                            # --- stage-2 (Hadamard + exp) for the PREVIOUS
                            # iteration: keeps every engine FIFO free of
                            # head-of-line waits on just-issued matmuls
                            inst = sc * NPAIR + pair
                            stage2.append((inst, bankM, bankX, oht, pair))
                            if len(stage2) > 1:
                                _emit_stage2(stage2.pop(0))
                            if len(pending) > 2:
                                _drain(_emit_zred(pending.pop(0)))

                for item in stage2:
                    _emit_stage2(item)
                for item in pending:
                    _drain(_emit_zred(item))
                for i, zt in enumerate(ztiles):
                    if i not in drained:
                        w = min(512, nsc * NPAIR * 16 - i * 512)
                        nc.vector.tensor_copy(zsb[:, i * 512:i * 512 + w],
                                              zt[:, :w])
                nc.sync.dma_start(zout[:], zsb[:])

    nc.compile()
    return nc


# ---------------------------------------------------------------- host side --

def _prep_host(X, eps, W1, b1, W2, b2, indices, nsc=BPC // SC, ncores=NCORES):
    """Build per-core input dicts + closures for unscrambling."""
    B = nsc * SC
    bf = np.float16
    # X: permute columns group-major, cast bf16, block layout [nsc, G, SC, GS]
    Xp = np.ascontiguousarray(X[:, GROUP_IDX.reshape(-1)]).astype(bf)  # (BATCH, 1024)
    W1b = W1.astype(bf)                              # (g, gs, hid)
    W2m = np.ascontiguousarray(W2[:, :, :LAT]).astype(bf)
    W2v = np.ascontiguousarray(W2[:, :, LAT:]).astype(bf)
    b1f = b1.astype(np.float32)
    hb2v = np.zeros((NPAIR, 128), np.float32)
    for p in range(NPAIR):
        hb2v[p, :64] = 0.5 * b2[2 * p, LAT:]
        hb2v[p, 64:] = 0.5 * b2[2 * p + 1, LAT:]
    selm = np.zeros((2, 128, 4), np.float32)
    selm[0, :64, 0] = 1.0   # zM g0
    selm[0, 64:, 1] = 1.0   # zM g1
    selm[1, :64, 2] = 1.0   # zX g0
    selm[1, 64:, 3] = 1.0   # zX g1
    selb = selm.astype(bf)

    QUAD = 4 if nsc % 4 == 0 else 1
    nquad = nsc // QUAD
    QW = QUAD * SC
    in_maps = []
    for core in range(ncores):
        lo = core * B
        Xc = Xp[lo:lo + B].reshape(nquad, QW, G, GS)
        xt = np.ascontiguousarray(Xc.transpose(0, 2, 1, 3))      # (nq,G,QW,GS)
        idxc = indices[:, lo:lo + B]                             # (G, B)
        ohc = np.zeros((nquad, NPAIR, 128, QW), np.float32)
        ar = np.arange(LAT)
        for p in range(NPAIR):
            for i, g in enumerate((2 * p, 2 * p + 1)):
                ii = idxc[g].reshape(nquad, QW)                  # (nq, QW)
                m = (ii[:, None, :] == ar[None, :, None])        # (nq, 64, QW)
                ohc[:, p, 64 * i:64 * i + 64, :] = m
        in_maps.append({
            "xt": xt, "oh": ohc.astype(np.int8), "w1": W1b, "w2m": W2m, "w2v": W2v,
            "b1": b1f, "hb2v": hb2v, "sel": selb,
        })
    return in_maps


def _unscramble(zdev, nsc):
    """zdev: (128, nsc*NPAIR*16) f32 -> zM, zX each (G, nsc*SC)."""
    B = nsc * SC
    zr = zdev.reshape(128, nsc, NPAIR, 4, 4)       # p, sc, pair, c, q
    zM = np.zeros((G, B), np.float32)
    zX = np.zeros((G, B), np.float32)
    for pair in range(NPAIR):
        for q, (dst, g) in enumerate(((zM, 2 * pair), (zM, 2 * pair + 1),
                                      (zX, 2 * pair), (zX, 2 * pair + 1))):
            blk = zr[:, :, pair, :, q]             # (128, nsc, 4)
            dst[g] = blk.transpose(1, 2, 0).reshape(B)
    return zM, zX


_NC_CACHE = {}


def kernel(X, eps, W1, b1, W2, b2, indices):
    nsc = BPC // SC
    key = (nsc, NCORES)
    if key not in _NC_CACHE:
        _NC_CACHE[key] = build_program(nsc, NCORES)
    nc = _NC_CACHE[key]
    in_maps = _prep_host(X, eps, W1, b1, W2, b2, indices)
    res = bass_utils.run_bass_kernel_spmd(nc, in_maps, core_ids=list(range(NCORES)))

    z = np.zeros((G, BATCH), np.float32)
    B = nsc * SC
    for core in range(NCORES):
        lo = core * B
        zM, zX = _unscramble(res.results[core]["z"], nsc)
        idxc = indices[:, lo:lo + B]
        b2m_sel = np.take_along_axis(b2[:, :LAT], idxc, axis=1)
        z[:, lo:lo + B] = zM + b2m_sel + eps[:, lo:lo + B] * zX
    return z.astype(np.float32)



# revision 14
# speedup vs baseline: 1.7247x; 1.7247x over previous
"""EnVAE sampling kernel for 8x TRN2 NeuronCores.

Math (per group g, batch element b):
  Xg = X[:, g::8]                                      # (b, 128)
  h  = relu(Xg @ W1[g] + b1[g])                        # (b, 128)
  out= h @ W2[g] + b2[g]; means=out[:, :64]; lv=out[:, 64:]
  z  = means[b, i] + eps * exp(0.5 * lv[b, i]),  i = indices[g, b]

Strategy: the latent index i is known on the host, so per group we sort the
batch by i and pad each (group, latent, core) bucket to a uniform quota.
After sorting, i is piecewise-constant in runs, so the "compute all 64
means/logvars then select" step collapses into per-run matmuls with a [128,2]
stationary = the selected (W2m[:,i], W2v[:,i]) column pair, producing
(zM, zL) = (selected mean-part, selected logvar-part) directly.  The host
finishes: z = zM + b2m[i] + eps * exp(0.5*zL + 0.5*b2v[i]).

mm1 runs in fp8(e4m3) DoubleRow perf mode (contraction 128 = 2 k-tiles of
64), mm2 in fp16.  No onehot tensors, no device-side exp/Hadamard - the only
elementwise work on device is the relu and the psum->sbuf output copies.
"""

import hashlib
import numpy as np
import ml_dtypes

import concourse.bass as bass
import concourse.bacc as bacc
import concourse.mybir as mybir
from concourse import tile
from concourse import bass_utils

OBS = 1024
LAT = 64
G = 8
GS = 128
HID = 128
BATCH = 65536
NCORES = 8

BP = 8256                      # padded per-core batch (uniform bucket quotas)
CHUNKS = [512] * 16 + [64]     # per-core column chunks (psum tile widths)
NCHUNK = len(CHUNKS)           # 17
QW = 2048                      # X piece width (4 pieces cover 8192 cols)
NPIECE = 4
NINST = NCHUNK * G             # 136
NROUND = (NINST + 3) // 4      # 34 zsel psum rounds (4 slots / 1-bank tile)
NDGRP = (NROUND + 3) // 4      # 9 staging drain groups (4 rounds each)

FP8 = mybir.dt.float8e4
F16 = mybir.dt.float16
F32 = mybir.dt.float32
E4 = ml_dtypes.float8_e4m3

GROUP_IDX = np.stack([np.arange(n, OBS, G) for n in range(G)])  # (g, gs)

CHUNK_OFF = np.concatenate([[0], np.cumsum(CHUNKS)])


def emission_order():
    """(chunk, g) emission order: chunk-pairs of the same group share one
    2-bank hp psum tile and a single relu; tail chunks come last."""
    order = []
    for pair in range(8):
        for g in range(G):
            order.append((2 * pair, g))
            order.append((2 * pair + 1, g))
    for g in range(G):
        order.append((16, g))
    return order


# ------------------------------------------------------------------- plan --

def _plan(indices):
    """Uniform per-core bucket quotas + per-core element selection.

    Returns:
      quota: (G, LAT) int - per-core count for each (group, latent) bucket
      sel:   (NCORES, G, BP) int32 - original batch index at each slot
      mask:  (NCORES, G, BP) bool - slot holds a real (non-dummy) element
      runs:  list over g of list over chunk of [(l, s, e), ...] segments
             (identical for every core by construction)
    """
    quota = np.zeros((G, LAT), np.int64)
    sel = np.zeros((NCORES, G, BP), np.int32)
    mask = np.zeros((NCORES, G, BP), bool)
    for g in range(G):
        idg = indices[g]
        order = np.argsort(idg, kind="stable")
        counts = np.bincount(idg, minlength=LAT).astype(np.int64)
        P = -(-counts // NCORES)          # ceil(n/8)
        deficit = BP - int(P.sum())
        assert deficit >= 0, f"BP={BP} too small: need {P.sum()}"
        P[:deficit] += 1
        quota[g] = P
        starts = np.concatenate([[0], np.cumsum(counts)])
        boundaries = np.concatenate([[0], np.cumsum(P)])
        for l in range(LAT):
            n, p = int(counts[l]), int(P[l])
            bucket = order[starts[l]:starts[l] + n]
            j = np.arange(p)
            for c in range(NCORES):
                pos = c * p + j
                real = pos < n
                sl = slice(boundaries[l], boundaries[l] + p)
                sel[c, g, sl] = bucket[np.minimum(pos, n - 1)]
                mask[c, g, sl] = real
    # run segments per (g, chunk): bucket boundaries cut by chunk edges
    runs = []
    for g in range(G):
        bnd = np.cumsum(quota[g])         # bucket end positions (last = BP)
        gruns = []
        for c in range(NCHUNK):
            c0, c1 = int(CHUNK_OFF[c]), int(CHUNK_OFF[c + 1])
            segs = []
            lo = c0
            for l in range(LAT):
                hi = int(bnd[l])
                if hi <= lo:
                    continue
                if lo >= c1:
                    break
                e = min(hi, c1)
                segs.append((l, lo - c0, e - c0))
                lo = e
            gruns.append(segs)
        runs.append(gruns)
    return quota, sel, mask, runs


# ---------------------------------------------------------------- program --

BUILD_CFG = {"relu_split": False, "defer": 6, "copy_first": False,
             "zsel_bufs": 2, "hp_bufs": 3, "hs_bufs": 8, "stg_bufs": 4,
             "zout_q": ("pool", "act")}


def build_program(runs, num_devices=NCORES):
    nc = bacc.Bacc("TRN2", target_bir_lowering=False, debug=False,
                   num_devices=num_devices)

    xq = nc.dram_tensor("xq", [NPIECE, G, 64, 2, QW], FP8,
                        kind="ExternalInput").ap()
    xtail = nc.dram_tensor("xtail", [G, 64, 2, 64], FP8,
                           kind="ExternalInput").ap()
    w1 = nc.dram_tensor("w1", [G, 64, 2, HID], FP8, kind="ExternalInput").ap()
    w2 = nc.dram_tensor("w2", [G, HID, LAT, 2], F16, kind="ExternalInput").ap()
    b1 = nc.dram_tensor("b1", [G, HID], F32, kind="ExternalInput").ap()
    # z: (drain grp, parity j, slot k, round blk, col); j=0 -> zM, j=1 -> zL
    zout = nc.dram_tensor("z", [NDGRP, 2, 4, 4, 512], F32,
                          kind="ExternalOutput").ap()

    from contextlib import ExitStack
    with tile.TileContext(nc) as tc, ExitStack() as st:
        cp = st.enter_context(tc.tile_pool(name="const", bufs=1))
        w1_sb = cp.tile([64, G, 2, HID], FP8, tag="w1")
        nc.sync.dma_start(w1_sb[:], w1.rearrange("g p t m -> p g t m"))
        b1_sb = cp.tile([HID, G], F32, tag="b1")
        nc.sync.dma_start(b1_sb[:], b1.rearrange("g k -> k g"))
        w2_sb = cp.tile([HID, G, LAT, 2], F16, tag="w2")
        xtl = cp.tile([64, G, 2, 64], FP8, tag="xtl")

        xpool = st.enter_context(tc.tile_pool(name="xg", bufs=20))
        hppool = st.enter_context(tc.tile_pool(name="hp",
                                               bufs=BUILD_CFG["hp_bufs"],
                                               space="PSUM"))
        hspool = st.enter_context(tc.tile_pool(name="hs",
                                               bufs=BUILD_CFG["hs_bufs"]))
        zpool = st.enter_context(tc.tile_pool(name="zp",
                                              bufs=BUILD_CFG["zsel_bufs"],
                                              space="PSUM"))
        spool = st.enter_context(tc.tile_pool(name="stg",
                                              bufs=BUILD_CFG["stg_bufs"]))

        # relu/copies run on ACT or DVE (GPSIMD cannot touch PSUM);
        # pick by accumulated load so the faster ACT takes a larger share.
        eng_load = {"act": 0.0, "dve": 0.0}

        def pick_engine(cost_act, cost_dve):
            if eng_load["act"] + cost_act <= eng_load["dve"] + cost_dve:
                eng_load["act"] += cost_act
                return "act"
            eng_load["dve"] += cost_dve
            return "dve"

        def _relu_one(e, dst, src, g):
            if e == "act":
                nc.scalar.activation(dst, src,
                                     mybir.ActivationFunctionType.Relu,
                                     bias=b1_sb[:, g:g + 1], scale=1.0)
            else:
                nc.vector.tensor_scalar(dst, src, b1_sb[:, g:g + 1], 0.0,
                                        mybir.AluOpType.add,
                                        mybir.AluOpType.max)

        def emit_relu(hs_t, hp_t, g, width):
            e = pick_engine(width * 0.833 + 143, width * 1.042 + 125)
            _relu_one(e, hs_t[:, :width], hp_t[:, :width], g)

        def emit_copy(dst, src):
            e = pick_engine(570, 658)
            if e == "act":
                nc.scalar.copy(dst, src)
            else:
                nc.vector.tensor_copy(dst, src)

        state = {"zt": None, "stg": None}
        pending_mm2 = []

        def emit_mm2(item):
            # item = (em, hs_t, base, chunk, g): zsel slot em -> round r =
            # em//4 (one [128,512] 1-bank tile = 4 partition-offset slots)
            em, hs_t, base, chunk, g = item
            rnd = em // 4
            off = 32 * (em % 4)
            if em % 4 == 0:
                state["zt"] = zpool.tile([128, 512], F32, name=f"zt{rnd}",
                                         tag="zsel")
            zt = state["zt"]
            segs = runs[g][chunk]
            for si, (l, s, e) in enumerate(segs):
                nc.tensor.matmul(zt[off:off + 2, s:e],
                                 w2_sb[:, g, l], hs_t[:, base + s:base + e],
                                 start=(si == 0), stop=(si == len(segs) - 1),
                                 skip_group_check=True,
                                 tile_position=(0, off))
            if em % 4 == 3 or em == NINST - 1:
                grp, blk = rnd // 4, rnd % 4
                if blk == 0:
                    state["stg"] = spool.tile([128, 4, 512], F32,
                                              name=f"stg{grp}", tag="stg")
                stg = state["stg"]
                emit_copy(stg[0:98, blk, :], zt[0:98, :])
                if blk == 3 or rnd == NROUND - 1:
                    nb = blk + 1
                    QS = {"pool": nc.gpsimd, "act": nc.scalar,
                          "dve": nc.vector, "sp": nc.sync}
                    q0, q1 = BUILD_CFG["zout_q"]
                    QS[q0].dma_start(zout[grp, 0, :, 0:nb, :],
                                     stg[0::32, 0:nb, :])
                    QS[q1].dma_start(zout[grp, 1, :, 0:nb, :],
                                     stg[1::32, 0:nb, :])

        em = 0
        order = emission_order()
        # X pieces: piece p covers chunks 4p..4p+3; set 0 loads upfront,
        # set p+1 streams in one DMA per pair while set p is consumed.
        xg = {}

        def load_piece(p, gg):
            t = xpool.tile([64, 2, QW], FP8, name=f"x{p}_{gg}", tag="xg")
            nc.sync.dma_start(t[:], xq[p, gg])
            xg[(p, gg)] = t

        load_piece(0, 0)
        load_piece(0, 1)
        # w2 is first needed by mm2 of pair 0 (~8us in); xtail only at the end
        nc.sync.dma_start(w2_sb[:], w2.rearrange("g k l j -> k g l j"))
        for gg in range(2, G):
            load_piece(0, gg)
        nc.sync.dma_start(xtl[:], xtail.rearrange("g p t m -> p g t m"))
        # pairs: (chunk, chunk+1) of one group share an hp/hs pair-tile
        i = 0
        while i < len(order):
            chunk, g = order[i]
            if chunk < 16:
                piece = chunk // 4
                if chunk % 4 == 0 and piece + 1 < NPIECE:
                    load_piece(piece + 1, g)
                c0, c1 = chunk, chunk + 1
                so0 = int(CHUNK_OFF[c0] % QW)
                so1 = int(CHUNK_OFF[c1] % QW)
                hp = hppool.tile([128, 1024], F32, tag="hp")
                nc.tensor.matmul(hp[:, 0:512], w1_sb[:, g],
                                 xg[(piece, g)][:, :, so0:so0 + 512],
                                 start=True, stop=True,
                                 perf_mode=mybir.MatmulPerfMode.DoubleRow)
                nc.tensor.matmul(hp[:, 512:1024], w1_sb[:, g],
                                 xg[(piece, g)][:, :, so1:so1 + 512],
                                 start=True, stop=True,
                                 perf_mode=mybir.MatmulPerfMode.DoubleRow)
                if BUILD_CFG["copy_first"]:
                    while len(pending_mm2) > BUILD_CFG["defer"]:
                        emit_mm2(pending_mm2.pop(0))
                hs_t = hspool.tile([128, 1024], F16, tag="hs")
                emit_relu(hs_t, hp, g, 1024)
                if not BUILD_CFG["copy_first"]:
                    while len(pending_mm2) > BUILD_CFG["defer"]:
                        emit_mm2(pending_mm2.pop(0))
                pending_mm2.append((em, hs_t, 0, c0, g))
                pending_mm2.append((em + 1, hs_t, 512, c1, g))
                em += 2
                i += 2
            else:
                hp = hppool.tile([128, 1024], F32, tag="hp")
                nc.tensor.matmul(hp[:, 0:64], w1_sb[:, g], xtl[:, g],
                                 start=True, stop=True,
                                 perf_mode=mybir.MatmulPerfMode.DoubleRow)
                hs_t = hspool.tile([128, 1024], F16, tag="hs")
                emit_relu(hs_t, hp, g, 64)
                while len(pending_mm2) > BUILD_CFG["defer"]:
                    emit_mm2(pending_mm2.pop(0))
                pending_mm2.append((em, hs_t, 0, chunk, g))
                em += 1
                i += 1
        for item in pending_mm2:
            emit_mm2(item)

    nc.compile()
    return nc


# ------------------------------------------------------------------- host --

def _prep_inputs(X, W1, b1, W2, sel):
    """Per-core input dicts (xq/xtail/w1/w2/b1)."""
    w1_dev = np.ascontiguousarray(
        W1.astype(E4).reshape(G, 2, 64, HID).transpose(0, 2, 1, 3))
    # w2 packed (g, k, l, j): j=0 -> mean col l, j=1 -> logvar col l
    w2_dev = np.ascontiguousarray(
        W2.astype(np.float16).reshape(G, HID, 2, LAT).transpose(0, 1, 3, 2))
    b1_dev = b1.astype(np.float32)
    in_maps = []
    for c in range(NCORES):
        xq_c = np.empty((NPIECE, G, 64, 2, QW), E4)
        xtail_c = np.empty((G, 64, 2, 64), E4)
        for g in range(G):
            Xc = X[sel[c, g]][:, GROUP_IDX[g]].astype(E4)   # (BP, 128)
            Xt = np.ascontiguousarray(Xc.T)                  # (128, BP)
            blk = Xt.reshape(2, 64, BP)                      # (t, p, col)
            xq_c[:, g] = (blk[:, :, :NPIECE * QW]
                          .reshape(2, 64, NPIECE, QW).transpose(2, 1, 0, 3))
            xtail_c[g] = blk[:, :, NPIECE * QW:].transpose(1, 0, 2)
        in_maps.append({"xq": xq_c, "xtail": xtail_c, "w1": w1_dev,
                        "w2": w2_dev, "b1": b1_dev})
    return in_maps


def _decode(zres):
    """(NDGRP,2,4,4,512) device output -> zM, zL each (G, BP)."""
    zM = np.empty((G, BP), np.float32)
    zL = np.empty((G, BP), np.float32)
    for em, (chunk, g) in enumerate(emission_order()):
        rnd = em // 4
        grp, blk = rnd // 4, rnd % 4
        k = em % 4
        c0, c1 = CHUNK_OFF[chunk], CHUNK_OFF[chunk + 1]
        w = c1 - c0
        zM[g, c0:c1] = zres[grp, 0, k, blk, :w]
        zL[g, c0:c1] = zres[grp, 1, k, blk, :w]
    return zM, zL


_NC_CACHE = {}


def kernel(X, eps, W1, b1, W2, b2, indices):
    X = np.asarray(X, np.float32)
    eps = np.asarray(eps, np.float32)
    W1 = np.asarray(W1, np.float32)
    b1 = np.asarray(b1, np.float32)
    W2 = np.asarray(W2, np.float32)
    b2 = np.asarray(b2, np.float32)
    indices = np.asarray(indices, np.int32)

    key = hashlib.sha256(indices.tobytes()).hexdigest()
    if key not in _NC_CACHE:
        quota, sel, mask, runs = _plan(indices)
        nc = build_program(runs, NCORES)
        _NC_CACHE.clear()
        _NC_CACHE[key] = (nc, sel, mask)
    nc, sel, mask = _NC_CACHE[key]

    in_maps = _prep_inputs(X, W1, b1, W2, sel)
    res = bass_utils.run_bass_kernel_spmd(nc, in_maps,
                                          core_ids=list(range(NCORES)))

    z = np.zeros((G, BATCH), np.float32)
    for c in range(NCORES):
        zM, zL = _decode(res.results[c]["z"])
        for g in range(G):
            m = mask[c, g]
            borig = sel[c, g][m]
            ig = indices[g, borig]
            zz = (zM[g][m] + b2[g, ig]
                  + eps[g, borig] * np.exp(0.5 * zL[g][m] + 0.5 * b2[g, LAT + ig]))
            z[g, borig] = zz
    return z.astype(np.float32)


# revision 16
# speedup vs baseline: 1.7648x; 1.0233x over previous
"""EnVAE sampling kernel for 8x TRN2 NeuronCores.

Math (per group g, batch element b):
  Xg = X[:, g::8]                                      # (b, 128)
  h  = relu(Xg @ W1[g] + b1[g])                        # (b, 128)
  out= h @ W2[g] + b2[g]; means=out[:, :64]; lv=out[:, 64:]
  z  = means[b, i] + eps * exp(0.5 * lv[b, i]),  i = indices[g, b]

Strategy: the latent index i is known on the host, so per group we sort the
batch by i and pad each (group, latent, core) bucket to a uniform quota.
After sorting, i is piecewise-constant in runs, so the "compute all 64
means/logvars then select" step collapses into per-run matmuls with a [128,2]
stationary = the selected (W2m[:,i], W2v[:,i]) column pair, producing
(zM, zL) = (selected mean-part, selected logvar-part) directly.  The host
finishes: z = zM + b2m[i] + eps * exp(0.5*zL + 0.5*b2v[i]).

mm1 runs in fp8(e4m3) DoubleRow perf mode (contraction 128 = 2 k-tiles of
64), mm2 in fp16.  No onehot tensors, no device-side exp/Hadamard - the only
elementwise work on device is the relu and the psum->sbuf output copies.
"""

import hashlib
import numpy as np
import ml_dtypes

import concourse.bass as bass
import concourse.bacc as bacc
import concourse.mybir as mybir
from concourse import tile
from concourse import bass_utils

OBS = 1024
LAT = 64
G = 8
GS = 128
HID = 128
BATCH = 65536
NCORES = 8

BP = 8256                      # padded per-core batch (uniform bucket quotas)
CHUNKS = [512] * 16 + [64]     # per-core column chunks (psum tile widths)
NCHUNK = len(CHUNKS)           # 17
QW = 2048                      # X piece width (4 pieces cover 8192 cols)
NPIECE = 4
NINST = NCHUNK * G             # 136
NROUND = (NINST + 3) // 4      # 34 zsel psum rounds (4 slots / 1-bank tile)
NDGRP = (NROUND + 3) // 4      # 9 staging drain groups (4 rounds each)

FP8 = mybir.dt.float8e4
F16 = mybir.dt.float16
F32 = mybir.dt.float32
E4 = ml_dtypes.float8_e4m3

GROUP_IDX = np.stack([np.arange(n, OBS, G) for n in range(G)])  # (g, gs)

CHUNK_OFF = np.concatenate([[0], np.cumsum(CHUNKS)])


def emission_order():
    """(chunk, g) emission order: chunk-pairs of the same group share one
    2-bank hp psum tile and a single relu; tail chunks come last."""
    order = []
    for pair in range(8):
        for g in range(G):
            order.append((2 * pair, g))
            order.append((2 * pair + 1, g))
    for g in range(G):
        order.append((16, g))
    return order


# ------------------------------------------------------------------- plan --

def _plan(indices):
    """Uniform per-core bucket quotas + per-core element selection.

    Returns:
      quota: (G, LAT) int - per-core count for each (group, latent) bucket
      sel:   (NCORES, G, BP) int32 - original batch index at each slot
      mask:  (NCORES, G, BP) bool - slot holds a real (non-dummy) element
      runs:  list over g of list over chunk of [(l, s, e), ...] segments
             (identical for every core by construction)
    """
    quota = np.zeros((G, LAT), np.int64)
    sel = np.zeros((NCORES, G, BP), np.int32)
    mask = np.zeros((NCORES, G, BP), bool)
    for g in range(G):
        idg = indices[g]
        order = np.argsort(idg, kind="stable")
        counts = np.bincount(idg, minlength=LAT).astype(np.int64)
        P = -(-counts // NCORES)          # ceil(n/8)
        deficit = BP - int(P.sum())
        assert deficit >= 0, f"BP={BP} too small: need {P.sum()}"
        P[:deficit] += 1
        quota[g] = P
        starts = np.concatenate([[0], np.cumsum(counts)])
        boundaries = np.concatenate([[0], np.cumsum(P)])
        for l in range(LAT):
            n, p = int(counts[l]), int(P[l])
            bucket = order[starts[l]:starts[l] + n]
            j = np.arange(p)
            for c in range(NCORES):
                pos = c * p + j
                real = pos < n
                sl = slice(boundaries[l], boundaries[l] + p)
                sel[c, g, sl] = bucket[np.minimum(pos, n - 1)]
                mask[c, g, sl] = real
    # run segments per (g, chunk): bucket boundaries cut by chunk edges
    runs = []
    for g in range(G):
        bnd = np.cumsum(quota[g])         # bucket end positions (last = BP)
        gruns = []
        for c in range(NCHUNK):
            c0, c1 = int(CHUNK_OFF[c]), int(CHUNK_OFF[c + 1])
            segs = []
            lo = c0
            for l in range(LAT):
                hi = int(bnd[l])
                if hi <= lo:
                    continue
                if lo >= c1:
                    break
                e = min(hi, c1)
                segs.append((l, lo - c0, e - c0))
                lo = e
            gruns.append(segs)
        runs.append(gruns)
    return quota, sel, mask, runs


# ---------------------------------------------------------------- program --

BUILD_CFG = {"relu_split": False, "defer": 4, "copy_first": False,
             "zsel_bufs": 2, "hp_bufs": 3, "hs_bufs": 8, "stg_bufs": 4,
             "zout_q": ("pool", "act")}


def build_program(runs, num_devices=NCORES):
    nc = bacc.Bacc("TRN2", target_bir_lowering=False, debug=False,
                   num_devices=num_devices)

    xq = nc.dram_tensor("xq", [NPIECE, G, 64, 2, QW], FP8,
                        kind="ExternalInput").ap()
    xtail = nc.dram_tensor("xtail", [G, 64, 2, 64], FP8,
                           kind="ExternalInput").ap()
    w1 = nc.dram_tensor("w1", [G, 64, 2, HID], FP8, kind="ExternalInput").ap()
    w2 = nc.dram_tensor("w2", [G, HID, LAT, 2], F16, kind="ExternalInput").ap()
    b1 = nc.dram_tensor("b1", [G, HID], F32, kind="ExternalInput").ap()
    # z: (drain grp, parity j, slot k, round blk, col); j=0 -> zM, j=1 -> zL
    zout = nc.dram_tensor("z", [NDGRP, 2, 4, 4, 512], F32,
                          kind="ExternalOutput").ap()

    from contextlib import ExitStack
    with tile.TileContext(nc) as tc, ExitStack() as st:
        cp = st.enter_context(tc.tile_pool(name="const", bufs=1))
        w1_sb = cp.tile([64, G, 2, HID], FP8, tag="w1")
        nc.sync.dma_start(w1_sb[:], w1.rearrange("g p t m -> p g t m"))
        b1_sb = cp.tile([HID, G], F32, tag="b1")
        w2_sb = cp.tile([HID, G, LAT, 2], F16, tag="w2")
        xtl = cp.tile([64, G, 2, 64], FP8, tag="xtl")

        xpool = st.enter_context(tc.tile_pool(name="xg", bufs=20))
        hppool = st.enter_context(tc.tile_pool(name="hp",
                                               bufs=BUILD_CFG["hp_bufs"],
                                               space="PSUM"))
        hspool = st.enter_context(tc.tile_pool(name="hs",
                                               bufs=BUILD_CFG["hs_bufs"]))
        zpool = st.enter_context(tc.tile_pool(name="zp",
                                              bufs=BUILD_CFG["zsel_bufs"],
                                              space="PSUM"))
        spool = st.enter_context(tc.tile_pool(name="stg",
                                              bufs=BUILD_CFG["stg_bufs"]))

        # relu/copies run on ACT or DVE (GPSIMD cannot touch PSUM);
        # pick by accumulated load so the faster ACT takes a larger share.
        eng_load = {"act": 0.0, "dve": 0.0}

        def pick_engine(cost_act, cost_dve):
            if eng_load["act"] + cost_act <= eng_load["dve"] + cost_dve:
                eng_load["act"] += cost_act
                return "act"
            eng_load["dve"] += cost_dve
            return "dve"

        def _relu_one(e, dst, src, g):
            if e == "act":
                nc.scalar.activation(dst, src,
                                     mybir.ActivationFunctionType.Relu,
                                     bias=b1_sb[:, g:g + 1], scale=1.0)
            else:
                nc.vector.tensor_scalar(dst, src, b1_sb[:, g:g + 1], 0.0,
                                        mybir.AluOpType.add,
                                        mybir.AluOpType.max)

        def emit_relu(hs_t, hp_t, g, width):
            e = pick_engine(width * 0.833 + 143, width * 1.042 + 125)
            _relu_one(e, hs_t[:, :width], hp_t[:, :width], g)

        def emit_copy(dst, src):
            e = pick_engine(570, 658)
            if e == "act":
                nc.scalar.copy(dst, src)
            else:
                nc.vector.tensor_copy(dst, src)

        state = {"zt": None, "stg": None}
        pending_mm2 = []

        def emit_mm2(item):
            # item = (em, hs_t, base, chunk, g): zsel slot em -> round r =
            # em//4 (one [128,512] 1-bank tile = 4 partition-offset slots)
            em, hs_t, base, chunk, g = item
            rnd = em // 4
            off = 32 * (em % 4)
            if em % 4 == 0:
                state["zt"] = zpool.tile([128, 512], F32, name=f"zt{rnd}",
                                         tag="zsel")
            zt = state["zt"]
            segs = runs[g][chunk]
            for si, (l, s, e) in enumerate(segs):
                nc.tensor.matmul(zt[off:off + 2, s:e],
                                 w2_sb[:, g, l], hs_t[:, base + s:base + e],
                                 start=(si == 0), stop=(si == len(segs) - 1),
                                 skip_group_check=True,
                                 tile_position=(0, off))
            if em % 4 == 3 or em == NINST - 1:
                grp, blk = rnd // 4, rnd % 4
                if blk == 0:
                    state["stg"] = spool.tile([128, 4, 512], F32,
                                              name=f"stg{grp}", tag="stg")
                stg = state["stg"]
                emit_copy(stg[0:98, blk, :], zt[0:98, :])
                if blk == 3 or rnd == NROUND - 1:
                    nb = blk + 1
                    QS = {"pool": nc.gpsimd, "act": nc.scalar,
                          "dve": nc.vector, "sp": nc.sync}
                    q0, q1 = BUILD_CFG["zout_q"]
                    QS[q0].dma_start(zout[grp, 0, :, 0:nb, :],
                                     stg[0::32, 0:nb, :])
                    QS[q1].dma_start(zout[grp, 1, :, 0:nb, :],
                                     stg[1::32, 0:nb, :])

        em = 0
        order = emission_order()
        # X pieces: piece p covers chunks 4p..4p+3; set 0 loads upfront,
        # set p+1 streams in one DMA per pair while set p is consumed.
        xg = {}

        def load_piece(p, gg):
            t = xpool.tile([64, 2, QW], FP8, name=f"x{p}_{gg}", tag="xg")
            nc.sync.dma_start(t[:], xq[p, gg])
            xg[(p, gg)] = t

        load_piece(0, 0)
        nc.sync.dma_start(b1_sb[:], b1.rearrange("g k -> k g"))
        load_piece(0, 1)
        # w2 is first needed by mm2 of pair 0 (~8us in); xtail only at the end
        nc.sync.dma_start(w2_sb[:], w2.rearrange("g k l j -> k g l j"))
        for gg in range(2, G):
            load_piece(0, gg)
        nc.sync.dma_start(xtl[:], xtail.rearrange("g p t m -> p g t m"))
        # pairs: (chunk, chunk+1) of one group share an hp/hs pair-tile
        i = 0
        while i < len(order):
            chunk, g = order[i]
            if chunk < 16:
                piece = chunk // 4
                if chunk % 4 == 0 and piece + 1 < NPIECE:
                    load_piece(piece + 1, g)
                c0, c1 = chunk, chunk + 1
                so0 = int(CHUNK_OFF[c0] % QW)
                so1 = int(CHUNK_OFF[c1] % QW)
                hp = hppool.tile([128, 1024], F32, tag="hp")
                nc.tensor.matmul(hp[:, 0:512], w1_sb[:, g],
                                 xg[(piece, g)][:, :, so0:so0 + 512],
                                 start=True, stop=True,
                                 perf_mode=mybir.MatmulPerfMode.DoubleRow)
                nc.tensor.matmul(hp[:, 512:1024], w1_sb[:, g],
                                 xg[(piece, g)][:, :, so1:so1 + 512],
                                 start=True, stop=True,
                                 perf_mode=mybir.MatmulPerfMode.DoubleRow)
                if BUILD_CFG["copy_first"]:
                    while len(pending_mm2) > BUILD_CFG["defer"]:
                        emit_mm2(pending_mm2.pop(0))
                hs_t = hspool.tile([128, 1024], F16, tag="hs")
                emit_relu(hs_t, hp, g, 1024)
                if not BUILD_CFG["copy_first"]:
                    while len(pending_mm2) > BUILD_CFG["defer"]:
                        emit_mm2(pending_mm2.pop(0))
                pending_mm2.append((em, hs_t, 0, c0, g))
                pending_mm2.append((em + 1, hs_t, 512, c1, g))
                em += 2
                i += 2
            else:
                hp = hppool.tile([128, 1024], F32, tag="hp")
                nc.tensor.matmul(hp[:, 0:64], w1_sb[:, g], xtl[:, g],
                                 start=True, stop=True,
                                 perf_mode=mybir.MatmulPerfMode.DoubleRow)
                hs_t = hspool.tile([128, 1024], F16, tag="hs")
                emit_relu(hs_t, hp, g, 64)
                while len(pending_mm2) > min(BUILD_CFG["defer"], 2):
                    emit_mm2(pending_mm2.pop(0))
                pending_mm2.append((em, hs_t, 0, chunk, g))
                em += 1
                i += 1
        for item in pending_mm2:
            emit_mm2(item)

    nc.compile()
    return nc


# ------------------------------------------------------------------- host --

def _prep_inputs(X, W1, b1, W2, sel):
    """Per-core input dicts (xq/xtail/w1/w2/b1)."""
    w1_dev = np.ascontiguousarray(
        W1.astype(E4).reshape(G, 2, 64, HID).transpose(0, 2, 1, 3))
    # w2 packed (g, k, l, j): j=0 -> mean col l, j=1 -> logvar col l
    w2_dev = np.ascontiguousarray(
        W2.astype(np.float16).reshape(G, HID, 2, LAT).transpose(0, 1, 3, 2))
    b1_dev = b1.astype(np.float32)
    in_maps = []
    for c in range(NCORES):
        xq_c = np.empty((NPIECE, G, 64, 2, QW), E4)
        xtail_c = np.empty((G, 64, 2, 64), E4)
        for g in range(G):
            Xc = X[sel[c, g]][:, GROUP_IDX[g]].astype(E4)   # (BP, 128)
            Xt = np.ascontiguousarray(Xc.T)                  # (128, BP)
            blk = Xt.reshape(2, 64, BP)                      # (t, p, col)
            xq_c[:, g] = (blk[:, :, :NPIECE * QW]
                          .reshape(2, 64, NPIECE, QW).transpose(2, 1, 0, 3))
            xtail_c[g] = blk[:, :, NPIECE * QW:].transpose(1, 0, 2)
        in_maps.append({"xq": xq_c, "xtail": xtail_c, "w1": w1_dev,
                        "w2": w2_dev, "b1": b1_dev})
    return in_maps


def _decode(zres):
    """(NDGRP,2,4,4,512) device output -> zM, zL each (G, BP)."""
    zM = np.empty((G, BP), np.float32)
    zL = np.empty((G, BP), np.float32)
    for em, (chunk, g) in enumerate(emission_order()):
        rnd = em // 4
        grp, blk = rnd // 4, rnd % 4
        k = em % 4
        c0, c1 = CHUNK_OFF[chunk], CHUNK_OFF[chunk + 1]
        w = c1 - c0
        zM[g, c0:c1] = zres[grp, 0, k, blk, :w]
        zL[g, c0:c1] = zres[grp, 1, k, blk, :w]
    return zM, zL


_NC_CACHE = {}


def kernel(X, eps, W1, b1, W2, b2, indices):
    X = np.asarray(X, np.float32)
    eps = np.asarray(eps, np.float32)
    W1 = np.asarray(W1, np.float32)
    b1 = np.asarray(b1, np.float32)
    W2 = np.asarray(W2, np.float32)
    b2 = np.asarray(b2, np.float32)
    indices = np.asarray(indices, np.int32)

    key = hashlib.sha256(indices.tobytes()).hexdigest()
    if key not in _NC_CACHE:
        quota, sel, mask, runs = _plan(indices)
        nc = build_program(runs, NCORES)
        _NC_CACHE.clear()
        _NC_CACHE[key] = (nc, sel, mask)
    nc, sel, mask = _NC_CACHE[key]

    in_maps = _prep_inputs(X, W1, b1, W2, sel)
    res = bass_utils.run_bass_kernel_spmd(nc, in_maps,
                                          core_ids=list(range(NCORES)))

    z = np.zeros((G, BATCH), np.float32)
    for c in range(NCORES):
        zM, zL = _decode(res.results[c]["z"])
        for g in range(G):
            m = mask[c, g]
            borig = sel[c, g][m]
            ig = indices[g, borig]
            zz = (zM[g][m] + b2[g, ig]
                  + eps[g, borig] * np.exp(0.5 * zL[g][m] + 0.5 * b2[g, LAT + ig]))
            z[g, borig] = zz
    return z.astype(np.float32)


# revision 20
# speedup vs baseline: 1.7823x; 1.0099x over previous
"""EnVAE sampling kernel for 8x TRN2 NeuronCores.

Math (per group g, batch element b):
  Xg = X[:, g::8]                                      # (b, 128)
  h  = relu(Xg @ W1[g] + b1[g])                        # (b, 128)
  out= h @ W2[g] + b2[g]; means=out[:, :64]; lv=out[:, 64:]
  z  = means[b, i] + eps * exp(0.5 * lv[b, i]),  i = indices[g, b]

Strategy: the latent index i is known on the host, so per group we sort the
batch by i and pad each (group, latent, core) bucket to a uniform quota.
After sorting, i is piecewise-constant in runs, so the "compute all 64
means/logvars then select" step collapses into per-run matmuls with a [128,2]
stationary = the selected (W2m[:,i], W2v[:,i]) column pair, producing
(zM, zL) = (selected mean-part, selected logvar-part) directly.  The host
finishes: z = zM + b2m[i] + eps * exp(0.5*zL + 0.5*b2v[i]).

mm1 runs in fp8(e4m3) DoubleRow perf mode (contraction 128 = 2 k-tiles of
64), mm2 in fp16.  No onehot tensors, no device-side exp/Hadamard - the only
elementwise work on device is the relu and the psum->sbuf output copies.
"""

import hashlib
import numpy as np
import ml_dtypes

import concourse.bass as bass
import concourse.bacc as bacc
import concourse.mybir as mybir
from concourse import tile
from concourse import bass_utils

OBS = 1024
LAT = 64
G = 8
GS = 128
HID = 128
BATCH = 65536
NCORES = 8

BP = 8256                      # padded per-core batch (uniform bucket quotas)
CHUNKS = [512] * 16 + [64]     # per-core column chunks (psum tile widths)
NCHUNK = len(CHUNKS)           # 17
QW = 2048                      # X piece width (4 pieces cover 8192 cols)
NPIECE = 4
NINST = NCHUNK * G             # 136
NROUND = (NINST + 3) // 4      # 34 zsel psum rounds (4 slots / 1-bank tile)
NDGRP = (NROUND + 3) // 4      # 9 staging drain groups (4 rounds each)

FP8 = mybir.dt.float8e4
F16 = mybir.dt.float16
F32 = mybir.dt.float32
E4 = ml_dtypes.float8_e4m3

GROUP_IDX = np.stack([np.arange(n, OBS, G) for n in range(G)])  # (g, gs)

CHUNK_OFF = np.concatenate([[0], np.cumsum(CHUNKS)])


def emission_order():
    """(chunk, g) emission order: chunk-pairs of the same group share one
    2-bank hp psum tile and a single relu; tail chunks come last."""
    order = []
    for pair in range(8):
        for g in range(G):
            order.append((2 * pair, g))
            order.append((2 * pair + 1, g))
    for g in range(G):
        order.append((16, g))
    return order


# ------------------------------------------------------------------- plan --

def _plan(indices):
    """Uniform per-core bucket quotas + per-core element selection.

    Returns:
      quota: (G, LAT) int - per-core count for each (group, latent) bucket
      sel:   (NCORES, G, BP) int32 - original batch index at each slot
      mask:  (NCORES, G, BP) bool - slot holds a real (non-dummy) element
      runs:  list over g of list over chunk of [(l, s, e), ...] segments
             (identical for every core by construction)
    """
    quota = np.zeros((G, LAT), np.int64)
    sel = np.zeros((NCORES, G, BP), np.int32)
    mask = np.zeros((NCORES, G, BP), bool)
    for g in range(G):
        idg = indices[g]
        order = np.argsort(idg, kind="stable")
        counts = np.bincount(idg, minlength=LAT).astype(np.int64)
        P = -(-counts // NCORES)          # ceil(n/8)
        deficit = BP - int(P.sum())
        assert deficit >= 0, f"BP={BP} too small: need {P.sum()}"
        P[:deficit] += 1
        quota[g] = P
        starts = np.concatenate([[0], np.cumsum(counts)])
        boundaries = np.concatenate([[0], np.cumsum(P)])
        for l in range(LAT):
            n, p = int(counts[l]), int(P[l])
            if n == 0:
                continue  # sel stays 0 / mask False; device output ignored
            bucket = order[starts[l]:starts[l] + n]
            j = np.arange(p)
            for c in range(NCORES):
                pos = c * p + j
                real = pos < n
                sl = slice(boundaries[l], boundaries[l] + p)
                sel[c, g, sl] = bucket[np.minimum(pos, n - 1)]
                mask[c, g, sl] = real
    # run segments per (g, chunk): bucket boundaries cut by chunk edges
    runs = []
    for g in range(G):
        bnd = np.cumsum(quota[g])         # bucket end positions (last = BP)
        gruns = []
        for c in range(NCHUNK):
            c0, c1 = int(CHUNK_OFF[c]), int(CHUNK_OFF[c + 1])
            segs = []
            lo = c0
            for l in range(LAT):
                hi = int(bnd[l])
                if hi <= lo:
                    continue
                if lo >= c1:
                    break
                e = min(hi, c1)
                segs.append((l, lo - c0, e - c0))
                lo = e
            gruns.append(segs)
        runs.append(gruns)
    return quota, sel, mask, runs


# ---------------------------------------------------------------- program --

BUILD_CFG = {"relu_split": False, "defer": 4, "copy_first": False,
             "zsel_bufs": 2, "hp_bufs": 3, "hs_bufs": 8, "stg_bufs": 6,
             "zout_q": ("pool", "act")}


def build_program(runs, num_devices=NCORES):
    nc = bacc.Bacc("TRN2", target_bir_lowering=False, debug=False,
                   num_devices=num_devices)

    xq = nc.dram_tensor("xq", [NPIECE, G, 64, 2, QW], FP8,
                        kind="ExternalInput").ap()
    xtail = nc.dram_tensor("xtail", [G, 64, 2, 64], FP8,
                           kind="ExternalInput").ap()
    w1 = nc.dram_tensor("w1", [G, 64, 2, HID], FP8, kind="ExternalInput").ap()
    w2 = nc.dram_tensor("w2", [G, HID, LAT, 2], F16, kind="ExternalInput").ap()
    b1 = nc.dram_tensor("b1", [G, HID], F32, kind="ExternalInput").ap()
    # z: (drain grp, psum row, round blk, col); slot k at rows 32k..32k+12:
    # run ri of the slot's (g,chunk) -> rows 32k+2ri (zM), 32k+2ri+1 (zL)
    zout = nc.dram_tensor("z", [NDGRP, 112, 4, 512], F16,
                          kind="ExternalOutput").ap()

    from contextlib import ExitStack
    with tile.TileContext(nc) as tc, ExitStack() as st:
        cp = st.enter_context(tc.tile_pool(name="const", bufs=1))
        w1_sb = cp.tile([64, G, 2, HID], FP8, tag="w1")
        nc.sync.dma_start(w1_sb[:], w1.rearrange("g p t m -> p g t m"))
        b1_sb = cp.tile([HID, G], F32, tag="b1")
        w2_sb = cp.tile([HID, G, LAT, 2], F16, tag="w2")
        xtl = cp.tile([64, G, 2, 64], FP8, tag="xtl")

        xpool = st.enter_context(tc.tile_pool(name="xg", bufs=33))
        hppool = st.enter_context(tc.tile_pool(name="hp",
                                               bufs=BUILD_CFG["hp_bufs"],
                                               space="PSUM"))
        hspool = st.enter_context(tc.tile_pool(name="hs",
                                               bufs=BUILD_CFG["hs_bufs"]))
        zpool = st.enter_context(tc.tile_pool(name="zp",
                                              bufs=BUILD_CFG["zsel_bufs"],
                                              space="PSUM"))
        spool = st.enter_context(tc.tile_pool(name="stg",
                                              bufs=BUILD_CFG["stg_bufs"]))

        # relu/copies run on ACT or DVE (GPSIMD cannot touch PSUM);
        # pick by accumulated load so the faster ACT takes a larger share.
        eng_load = {"act": 0.0, "dve": 0.0}

        def pick_engine(cost_act, cost_dve):
            if eng_load["act"] + cost_act <= eng_load["dve"] + cost_dve:
                eng_load["act"] += cost_act
                return "act"
            eng_load["dve"] += cost_dve
            return "dve"

        def _relu_one(e, dst, src, g):
            if e == "act":
                nc.scalar.activation(dst, src,
                                     mybir.ActivationFunctionType.Relu,
                                     bias=b1_sb[:, g:g + 1], scale=1.0)
            else:
                nc.vector.tensor_scalar(dst, src, b1_sb[:, g:g + 1], 0.0,
                                        mybir.AluOpType.add,
                                        mybir.AluOpType.max)

        def emit_relu(hs_t, hp_t, g, width):
            e = pick_engine(width * 0.833 + 143, width * 1.042 + 125)
            _relu_one(e, hs_t[:, :width], hp_t[:, :width], g)

        def emit_copy(dst, src):
            e = pick_engine(570, 658)
            if e == "act":
                nc.scalar.copy(dst, src)
            else:
                nc.vector.tensor_copy(dst, src)

        state = {"zt": None, "stg": None}
        pending_mm2 = []

        def emit_mm2(item):
            # item = (em, hs_t, base, chunk, g): zsel slot em -> round r =
            # em//4 (one [128,512] 1-bank tile = 4 partition-offset slots)
            em, hs_t, base, chunk, g = item
            rnd = em // 4
            off = 32 * (em % 4)
            if em % 4 == 0:
                state["zt"] = zpool.tile([128, 512], F32, name=f"zt{rnd}",
                                         tag="zsel")
            zt = state["zt"]
            segs = runs[g][chunk]
            width = CHUNK_OFF[chunk + 1] - CHUNK_OFF[chunk]
            nr = len(segs)
            assert nr <= 8, f"chunk spans {nr} runs"
            l0 = segs[0][0]
            assert segs[-1][0] == l0 + nr - 1, "runs not consecutive"
            nc.tensor.matmul(zt[off:off + 2 * nr, :width],
                             w2_sb[:, g, l0:l0 + nr],
                             hs_t[:, base:base + width],
                             start=True, stop=True,
                             skip_group_check=True,
                             tile_position=(0, off))
            if em % 4 == 3 or em == NINST - 1:
                grp, blk = rnd // 4, rnd % 4
                if blk == 0:
                    state["stg"] = spool.tile([128, 4, 512], F16,
                                              name=f"stg{grp}", tag="stg")
                stg = state["stg"]
                emit_copy(stg[0:112, blk, :], zt[0:112, :])
                if blk == 3 or rnd == NROUND - 1:
                    nb = blk + 1
                    QS = {"pool": nc.gpsimd, "act": nc.scalar,
                          "dve": nc.vector, "sp": nc.sync}
                    q0, q1 = BUILD_CFG["zout_q"]
                    half = 2 if nb > 1 else 1
                    QS[q0].dma_start(zout[grp, :, 0:half, :],
                                     stg[0:112, 0:half, :])
                    if nb > half:
                        QS[q1].dma_start(zout[grp, :, half:nb, :],
                                         stg[0:112, half:nb, :])

        em = 0
        order = emission_order()
        # X pieces: piece p covers chunks 4p..4p+3; set 0 loads upfront,
        # set p+1 streams in one DMA per pair while set p is consumed.
        xg = {}

        def load_piece(p, gg):
            t = xpool.tile([64, 2, QW], FP8, name=f"x{p}_{gg}", tag="xg")
            nc.sync.dma_start(t[:], xq[p, gg])
            xg[(p, gg)] = t

        load_piece(0, 0)
        nc.sync.dma_start(b1_sb[:], b1.rearrange("g k -> k g"))
        load_piece(0, 1)
        # w2 is first needed by mm2 of pair 0 (~8us in); xtail only at the end
        nc.sync.dma_start(w2_sb[:], w2.rearrange("g k l j -> k g l j"))
        for gg in range(2, G):
            load_piece(0, gg)
        nc.sync.dma_start(xtl[:], xtail.rearrange("g p t m -> p g t m"))
        # pairs: (chunk, chunk+1) of one group share an hp/hs pair-tile
        i = 0
        while i < len(order):
            chunk, g = order[i]
            if chunk < 16:
                piece = chunk // 4
                if chunk % 4 == 0 and piece + 1 < NPIECE:
                    load_piece(piece + 1, g)
                c0, c1 = chunk, chunk + 1
                so0 = int(CHUNK_OFF[c0] % QW)
                so1 = int(CHUNK_OFF[c1] % QW)
                hp = hppool.tile([128, 1024], F32, tag="hp")
                nc.tensor.matmul(hp[:, 0:512], w1_sb[:, g],
                                 xg[(piece, g)][:, :, so0:so0 + 512],
                                 start=True, stop=True,
                                 perf_mode=mybir.MatmulPerfMode.DoubleRow)
                nc.tensor.matmul(hp[:, 512:1024], w1_sb[:, g],
                                 xg[(piece, g)][:, :, so1:so1 + 512],
                                 start=True, stop=True,
                                 perf_mode=mybir.MatmulPerfMode.DoubleRow)
                if BUILD_CFG["copy_first"]:
                    while len(pending_mm2) > BUILD_CFG["defer"]:
                        emit_mm2(pending_mm2.pop(0))
                hs_t = hspool.tile([128, 1024], F16, tag="hs")
                emit_relu(hs_t, hp, g, 1024)
                if not BUILD_CFG["copy_first"]:
                    while len(pending_mm2) > BUILD_CFG["defer"]:
                        emit_mm2(pending_mm2.pop(0))
                pending_mm2.append((em, hs_t, 0, c0, g))
                pending_mm2.append((em + 1, hs_t, 512, c1, g))
                em += 2
                i += 2
            else:
                hp = hppool.tile([128, 1024], F32, tag="hp")
                nc.tensor.matmul(hp[:, 0:64], w1_sb[:, g], xtl[:, g],
                                 start=True, stop=True,
                                 perf_mode=mybir.MatmulPerfMode.DoubleRow)
                hs_t = hspool.tile([128, 1024], F16, tag="hs")
                emit_relu(hs_t, hp, g, 64)
                while len(pending_mm2) > min(BUILD_CFG["defer"], 2):
                    emit_mm2(pending_mm2.pop(0))
                pending_mm2.append((em, hs_t, 0, chunk, g))
                em += 1
                i += 1
        for item in pending_mm2:
            emit_mm2(item)

    nc.compile()
    return nc


# ------------------------------------------------------------------- host --

def _prep_inputs(X, W1, b1, W2, sel):
    """Per-core input dicts (xq/xtail/w1/w2/b1)."""
    w1_dev = np.ascontiguousarray(
        W1.astype(E4).reshape(G, 2, 64, HID).transpose(0, 2, 1, 3))
    # w2 packed (g, k, l, j): j=0 -> mean col l, j=1 -> logvar col l
    w2_dev = np.ascontiguousarray(
        W2.astype(np.float16).reshape(G, HID, 2, LAT).transpose(0, 1, 3, 2))
    b1_dev = b1.astype(np.float32)
    in_maps = []
    for c in range(NCORES):
        xq_c = np.empty((NPIECE, G, 64, 2, QW), E4)
        xtail_c = np.empty((G, 64, 2, 64), E4)
        for g in range(G):
            Xc = X[sel[c, g]][:, GROUP_IDX[g]].astype(E4)   # (BP, 128)
            Xt = np.ascontiguousarray(Xc.T)                  # (128, BP)
            blk = Xt.reshape(2, 64, BP)                      # (t, p, col)
            xq_c[:, g] = (blk[:, :, :NPIECE * QW]
                          .reshape(2, 64, NPIECE, QW).transpose(2, 1, 0, 3))
            xtail_c[g] = blk[:, :, NPIECE * QW:].transpose(1, 0, 2)
        in_maps.append({"xq": xq_c, "xtail": xtail_c, "w1": w1_dev,
                        "w2": w2_dev, "b1": b1_dev})
    return in_maps


def _decode(zres, runs):
    """(NDGRP,112,4,512) f16 device output -> zM, zL each (G, BP)."""
    zM = np.empty((G, BP), np.float32)
    zL = np.empty((G, BP), np.float32)
    for em, (chunk, g) in enumerate(emission_order()):
        rnd = em // 4
        grp, blk = rnd // 4, rnd % 4
        k = em % 4
        c0 = CHUNK_OFF[chunk]
        for ri, (l, s, e) in enumerate(runs[g][chunk]):
            zM[g, c0 + s:c0 + e] = zres[grp, 32 * k + 2 * ri, blk, s:e]
            zL[g, c0 + s:c0 + e] = zres[grp, 32 * k + 2 * ri + 1, blk, s:e]
    return zM, zL


_NC_CACHE = {}


def kernel(X, eps, W1, b1, W2, b2, indices):
    X = np.asarray(X, np.float32)
    eps = np.asarray(eps, np.float32)
    W1 = np.asarray(W1, np.float32)
    b1 = np.asarray(b1, np.float32)
    W2 = np.asarray(W2, np.float32)
    b2 = np.asarray(b2, np.float32)
    indices = np.asarray(indices, np.int32)

    key = hashlib.sha256(indices.tobytes()).hexdigest()
    if key not in _NC_CACHE:
        quota, sel, mask, runs = _plan(indices)
        nc = build_program(runs, NCORES)
        _NC_CACHE.clear()
        _NC_CACHE[key] = (nc, sel, mask, runs)
    nc, sel, mask, runs = _NC_CACHE[key]

    in_maps = _prep_inputs(X, W1, b1, W2, sel)
    res = bass_utils.run_bass_kernel_spmd(nc, in_maps,
                                          core_ids=list(range(NCORES)))

    z = np.zeros((G, BATCH), np.float32)
    for c in range(NCORES):
        zM, zL = _decode(res.results[c]["z"], runs)
        for g in range(G):
            m = mask[c, g]
            borig = sel[c, g][m]
            ig = indices[g, borig]
            zz = (zM[g][m] + b2[g, ig]
                  + eps[g, borig] * np.exp(0.5 * zL[g][m] + 0.5 * b2[g, LAT + ig]))
            z[g, borig] = zz
    return z.astype(np.float32)


# revision 21
# speedup vs baseline: 1.8014x; 1.0107x over previous
"""EnVAE sampling kernel for 8x TRN2 NeuronCores.

Math (per group g, batch element b):
  Xg = X[:, g::8]                                      # (b, 128)
  h  = relu(Xg @ W1[g] + b1[g])                        # (b, 128)
  out= h @ W2[g] + b2[g]; means=out[:, :64]; lv=out[:, 64:]
  z  = means[b, i] + eps * exp(0.5 * lv[b, i]),  i = indices[g, b]

Strategy: the latent index i is known on the host, so per group we sort the
batch by i and pad each (group, latent, core) bucket to a uniform quota.
After sorting, i is piecewise-constant in runs, so the "compute all 64
means/logvars then select" step collapses into per-run matmuls with a [128,2]
stationary = the selected (W2m[:,i], W2v[:,i]) column pair, producing
(zM, zL) = (selected mean-part, selected logvar-part) directly.  The host
finishes: z = zM + b2m[i] + eps * exp(0.5*zL + 0.5*b2v[i]).

mm1 runs in fp8(e4m3) DoubleRow perf mode (contraction 128 = 2 k-tiles of
64), mm2 in fp16.  No onehot tensors, no device-side exp/Hadamard - the only
elementwise work on device is the relu and the psum->sbuf output copies.
"""

import hashlib
import numpy as np
import ml_dtypes

import concourse.bass as bass
import concourse.bacc as bacc
import concourse.mybir as mybir
from concourse import tile
from concourse import bass_utils

OBS = 1024
LAT = 64
G = 8
GS = 128
HID = 128
BATCH = 65536
NCORES = 8

BP = 8256                      # padded per-core batch (uniform bucket quotas)
CHUNKS = [512] * 16 + [64]     # per-core column chunks (psum tile widths)
NCHUNK = len(CHUNKS)           # 17
QW = 2048                      # X piece width (4 pieces cover 8192 cols)
NPIECE = 4
NINST = NCHUNK * G             # 136
NROUND = (NINST + 3) // 4      # 34 zsel psum rounds (4 slots / 1-bank tile)
NDGRP = (NROUND + 3) // 4      # 9 staging drain groups (4 rounds each)

FP8 = mybir.dt.float8e4
F16 = mybir.dt.float16
F32 = mybir.dt.float32
E4 = ml_dtypes.float8_e4m3

GROUP_IDX = np.stack([np.arange(n, OBS, G) for n in range(G)])  # (g, gs)

CHUNK_OFF = np.concatenate([[0], np.cumsum(CHUNKS)])


def emission_order():
    """(chunk, g) emission order: chunk-pairs of the same group share one
    2-bank hp psum tile and a single relu; tail chunks come last."""
    order = []
    for pair in range(8):
        for g in range(G):
            order.append((2 * pair, g))
            order.append((2 * pair + 1, g))
    for g in range(G):
        order.append((16, g))
    return order


# ------------------------------------------------------------------- plan --

def _plan(indices):
    """Uniform per-core bucket quotas + per-core element selection.

    Returns:
      quota: (G, LAT) int - per-core count for each (group, latent) bucket
      sel:   (NCORES, G, BP) int32 - original batch index at each slot
      mask:  (NCORES, G, BP) bool - slot holds a real (non-dummy) element
      runs:  list over g of list over chunk of [(l, s, e), ...] segments
             (identical for every core by construction)
    """
    quota = np.zeros((G, LAT), np.int64)
    sel = np.zeros((NCORES, G, BP), np.int32)
    mask = np.zeros((NCORES, G, BP), bool)
    for g in range(G):
        idg = indices[g]
        order = np.argsort(idg, kind="stable")
        counts = np.bincount(idg, minlength=LAT).astype(np.int64)
        P = -(-counts // NCORES)          # ceil(n/8)
        deficit = BP - int(P.sum())
        assert deficit >= 0, f"BP={BP} too small: need {P.sum()}"
        P[:deficit] += 1
        quota[g] = P
        starts = np.concatenate([[0], np.cumsum(counts)])
        boundaries = np.concatenate([[0], np.cumsum(P)])
        for l in range(LAT):
            n, p = int(counts[l]), int(P[l])
            if n == 0:
                continue  # sel stays 0 / mask False; device output ignored
            bucket = order[starts[l]:starts[l] + n]
            j = np.arange(p)
            for c in range(NCORES):
                pos = c * p + j
                real = pos < n
                sl = slice(boundaries[l], boundaries[l] + p)
                sel[c, g, sl] = bucket[np.minimum(pos, n - 1)]
                mask[c, g, sl] = real
    # run segments per (g, chunk): bucket boundaries cut by chunk edges
    runs = []
    for g in range(G):
        bnd = np.cumsum(quota[g])         # bucket end positions (last = BP)
        gruns = []
        for c in range(NCHUNK):
            c0, c1 = int(CHUNK_OFF[c]), int(CHUNK_OFF[c + 1])
            segs = []
            lo = c0
            for l in range(LAT):
                hi = int(bnd[l])
                if hi <= lo:
                    continue
                if lo >= c1:
                    break
                e = min(hi, c1)
                segs.append((l, lo - c0, e - c0))
                lo = e
            gruns.append(segs)
        runs.append(gruns)
    return quota, sel, mask, runs


# ---------------------------------------------------------------- program --

BUILD_CFG = {"relu_split": False, "defer": 4, "copy_first": False,
             "zsel_bufs": 2, "hp_bufs": 3, "hs_bufs": 8, "stg_bufs": 6,
             "zout_q": ("act", "pool")}


def build_program(runs, num_devices=NCORES):
    nc = bacc.Bacc("TRN2", target_bir_lowering=False, debug=False,
                   num_devices=num_devices)

    xq = nc.dram_tensor("xq", [NPIECE, G, 64, 2, QW], FP8,
                        kind="ExternalInput").ap()
    xtail = nc.dram_tensor("xtail", [G, 64, 2, 64], FP8,
                           kind="ExternalInput").ap()
    w1 = nc.dram_tensor("w1", [G, 64, 2, HID], FP8, kind="ExternalInput").ap()
    w2 = nc.dram_tensor("w2", [G, HID, LAT, 2], F16, kind="ExternalInput").ap()
    b1 = nc.dram_tensor("b1", [G, HID], F32, kind="ExternalInput").ap()
    # z: (drain grp, psum row, round blk, col); slot k at rows 32k..32k+12:
    # run ri of the slot's (g,chunk) -> rows 32k+2ri (zM), 32k+2ri+1 (zL)
    zout = nc.dram_tensor("z", [NDGRP, 112, 4, 512], F16,
                          kind="ExternalOutput").ap()

    from contextlib import ExitStack
    with tile.TileContext(nc) as tc, ExitStack() as st:
        cp = st.enter_context(tc.tile_pool(name="const", bufs=1))
        w1_sb = cp.tile([64, G, 2, HID], FP8, tag="w1")
        nc.sync.dma_start(w1_sb[:], w1.rearrange("g p t m -> p g t m"))
        b1_sb = cp.tile([HID, G], F32, tag="b1")
        w2_sb = cp.tile([HID, G, LAT, 2], F16, tag="w2")
        xtl = cp.tile([64, G, 2, 64], FP8, tag="xtl")

        xpool = st.enter_context(tc.tile_pool(name="xg", bufs=33))
        hppool = st.enter_context(tc.tile_pool(name="hp",
                                               bufs=BUILD_CFG["hp_bufs"],
                                               space="PSUM"))
        hspool = st.enter_context(tc.tile_pool(name="hs",
                                               bufs=BUILD_CFG["hs_bufs"]))
        zpool = st.enter_context(tc.tile_pool(name="zp",
                                              bufs=BUILD_CFG["zsel_bufs"],
                                              space="PSUM"))
        spool = st.enter_context(tc.tile_pool(name="stg",
                                              bufs=BUILD_CFG["stg_bufs"]))

        # relu/copies run on ACT or DVE (GPSIMD cannot touch PSUM);
        # pick by accumulated load so the faster ACT takes a larger share.
        eng_load = {"act": 0.0, "dve": 0.0}

        def pick_engine(cost_act, cost_dve):
            if eng_load["act"] + cost_act <= eng_load["dve"] + cost_dve:
                eng_load["act"] += cost_act
                return "act"
            eng_load["dve"] += cost_dve
            return "dve"

        def _relu_one(e, dst, src, g):
            if e == "act":
                nc.scalar.activation(dst, src,
                                     mybir.ActivationFunctionType.Relu,
                                     bias=b1_sb[:, g:g + 1], scale=1.0)
            else:
                nc.vector.tensor_scalar(dst, src, b1_sb[:, g:g + 1], 0.0,
                                        mybir.AluOpType.add,
                                        mybir.AluOpType.max)

        def emit_relu(hs_t, hp_t, g, width):
            e = pick_engine(width * 0.833 + 143, width * 1.042 + 125)
            _relu_one(e, hs_t[:, :width], hp_t[:, :width], g)

        def emit_copy(dst, src):
            e = pick_engine(570, 658)
            if e == "act":
                nc.scalar.copy(dst, src)
            else:
                nc.vector.tensor_copy(dst, src)

        state = {"zt": None, "stg": None}
        pending_mm2 = []

        def emit_mm2(item):
            # item = (em, hs_t, base, chunk, g): zsel slot em -> round r =
            # em//4 (one [128,512] 1-bank tile = 4 partition-offset slots)
            em, hs_t, base, chunk, g = item
            rnd = em // 4
            off = 32 * (em % 4)
            if em % 4 == 0:
                state["zt"] = zpool.tile([128, 512], F32, name=f"zt{rnd}",
                                         tag="zsel")
            zt = state["zt"]
            segs = runs[g][chunk]
            width = CHUNK_OFF[chunk + 1] - CHUNK_OFF[chunk]
            nr = len(segs)
            assert nr <= 8, f"chunk spans {nr} runs"
            l0 = segs[0][0]
            assert segs[-1][0] == l0 + nr - 1, "runs not consecutive"
            nc.tensor.matmul(zt[off:off + 2 * nr, :width],
                             w2_sb[:, g, l0:l0 + nr],
                             hs_t[:, base:base + width],
                             start=True, stop=True,
                             skip_group_check=True,
                             tile_position=(0, off))
            if em % 4 == 3 or em == NINST - 1:
                grp, blk = rnd // 4, rnd % 4
                if blk == 0:
                    state["stg"] = spool.tile([128, 4, 512], F16,
                                              name=f"stg{grp}", tag="stg")
                stg = state["stg"]
                emit_copy(stg[0:112, blk, :], zt[0:112, :])
                if blk == 3 or rnd == NROUND - 1:
                    nb = blk + 1
                    QS = {"pool": nc.gpsimd, "act": nc.scalar,
                          "dve": nc.vector, "sp": nc.sync}
                    q0, q1 = BUILD_CFG["zout_q"]
                    half = 2 if nb > 1 else 1
                    QS[q0].dma_start(zout[grp, :, 0:half, :],
                                     stg[0:112, 0:half, :])
                    if nb > half:
                        QS[q1].dma_start(zout[grp, :, half:nb, :],
                                         stg[0:112, half:nb, :])

        em = 0
        order = emission_order()
        # X pieces: piece p covers chunks 4p..4p+3; set 0 loads upfront,
        # set p+1 streams in one DMA per pair while set p is consumed.
        xg = {}

        def load_piece(p, gg):
            t = xpool.tile([64, 2, QW], FP8, name=f"x{p}_{gg}", tag="xg")
            nc.sync.dma_start(t[:], xq[p, gg])
            xg[(p, gg)] = t

        load_piece(0, 0)
        nc.sync.dma_start(b1_sb[:], b1.rearrange("g k -> k g"))
        load_piece(0, 1)
        # w2 is first needed by mm2 of pair 0 (~8us in); xtail only at the end
        nc.sync.dma_start(w2_sb[:], w2.rearrange("g k l j -> k g l j"))
        for gg in range(2, G):
            load_piece(0, gg)
        nc.sync.dma_start(xtl[:], xtail.rearrange("g p t m -> p g t m"))
        # pairs: (chunk, chunk+1) of one group share an hp/hs pair-tile
        i = 0
        while i < len(order):
            chunk, g = order[i]
            if chunk < 16:
                piece = chunk // 4
                if chunk % 4 == 0 and piece + 1 < NPIECE:
                    load_piece(piece + 1, g)
                c0, c1 = chunk, chunk + 1
                so0 = int(CHUNK_OFF[c0] % QW)
                so1 = int(CHUNK_OFF[c1] % QW)
                hp = hppool.tile([128, 1024], F32, tag="hp")
                nc.tensor.matmul(hp[:, 0:512], w1_sb[:, g],
                                 xg[(piece, g)][:, :, so0:so0 + 512],
                                 start=True, stop=True,
                                 perf_mode=mybir.MatmulPerfMode.DoubleRow)
                nc.tensor.matmul(hp[:, 512:1024], w1_sb[:, g],
                                 xg[(piece, g)][:, :, so1:so1 + 512],
                                 start=True, stop=True,
                                 perf_mode=mybir.MatmulPerfMode.DoubleRow)
                if BUILD_CFG["copy_first"]:
                    while len(pending_mm2) > BUILD_CFG["defer"]:
                        emit_mm2(pending_mm2.pop(0))
                hs_t = hspool.tile([128, 1024], F16, tag="hs")
                emit_relu(hs_t, hp, g, 1024)
                if not BUILD_CFG["copy_first"]:
                    while len(pending_mm2) > BUILD_CFG["defer"]:
                        emit_mm2(pending_mm2.pop(0))
                pending_mm2.append((em, hs_t, 0, c0, g))
                pending_mm2.append((em + 1, hs_t, 512, c1, g))
                em += 2
                i += 2
            else:
                hp = hppool.tile([128, 1024], F32, tag="hp")
                nc.tensor.matmul(hp[:, 0:64], w1_sb[:, g], xtl[:, g],
                                 start=True, stop=True,
                                 perf_mode=mybir.MatmulPerfMode.DoubleRow)
                hs_t = hspool.tile([128, 1024], F16, tag="hs")
                emit_relu(hs_t, hp, g, 64)
                while len(pending_mm2) > min(BUILD_CFG["defer"], 2):
                    emit_mm2(pending_mm2.pop(0))
                pending_mm2.append((em, hs_t, 0, chunk, g))
                em += 1
                i += 1
        for item in pending_mm2:
            emit_mm2(item)

    nc.compile()
    return nc


# ------------------------------------------------------------------- host --

def _prep_inputs(X, W1, b1, W2, sel):
    """Per-core input dicts (xq/xtail/w1/w2/b1)."""
    w1_dev = np.ascontiguousarray(
        W1.astype(E4).reshape(G, 2, 64, HID).transpose(0, 2, 1, 3))
    # w2 packed (g, k, l, j): j=0 -> mean col l, j=1 -> logvar col l
    w2_dev = np.ascontiguousarray(
        W2.astype(np.float16).reshape(G, HID, 2, LAT).transpose(0, 1, 3, 2))
    b1_dev = b1.astype(np.float32)
    in_maps = []
    for c in range(NCORES):
        xq_c = np.empty((NPIECE, G, 64, 2, QW), E4)
        xtail_c = np.empty((G, 64, 2, 64), E4)
        for g in range(G):
            Xc = X[sel[c, g]][:, GROUP_IDX[g]].astype(E4)   # (BP, 128)
            Xt = np.ascontiguousarray(Xc.T)                  # (128, BP)
            blk = Xt.reshape(2, 64, BP)                      # (t, p, col)
            xq_c[:, g] = (blk[:, :, :NPIECE * QW]
                          .reshape(2, 64, NPIECE, QW).transpose(2, 1, 0, 3))
            xtail_c[g] = blk[:, :, NPIECE * QW:].transpose(1, 0, 2)
        in_maps.append({"xq": xq_c, "xtail": xtail_c, "w1": w1_dev,
                        "w2": w2_dev, "b1": b1_dev})
    return in_maps


def _decode(zres, runs):
    """(NDGRP,112,4,512) f16 device output -> zM, zL each (G, BP)."""
    zM = np.empty((G, BP), np.float32)
    zL = np.empty((G, BP), np.float32)
    for em, (chunk, g) in enumerate(emission_order()):
        rnd = em // 4
        grp, blk = rnd // 4, rnd % 4
        k = em % 4
        c0 = CHUNK_OFF[chunk]
        for ri, (l, s, e) in enumerate(runs[g][chunk]):
            zM[g, c0 + s:c0 + e] = zres[grp, 32 * k + 2 * ri, blk, s:e]
            zL[g, c0 + s:c0 + e] = zres[grp, 32 * k + 2 * ri + 1, blk, s:e]
    return zM, zL


_NC_CACHE = {}


def kernel(X, eps, W1, b1, W2, b2, indices):
    X = np.asarray(X, np.float32)
    eps = np.asarray(eps, np.float32)
    W1 = np.asarray(W1, np.float32)
    b1 = np.asarray(b1, np.float32)
    W2 = np.asarray(W2, np.float32)
    b2 = np.asarray(b2, np.float32)
    indices = np.asarray(indices, np.int32)

    key = hashlib.sha256(indices.tobytes()).hexdigest()
    if key not in _NC_CACHE:
        quota, sel, mask, runs = _plan(indices)
        nc = build_program(runs, NCORES)
        _NC_CACHE.clear()
        _NC_CACHE[key] = (nc, sel, mask, runs)
    nc, sel, mask, runs = _NC_CACHE[key]

    in_maps = _prep_inputs(X, W1, b1, W2, sel)
    res = bass_utils.run_bass_kernel_spmd(nc, in_maps,
                                          core_ids=list(range(NCORES)))

    z = np.zeros((G, BATCH), np.float32)
    for c in range(NCORES):
        zM, zL = _decode(res.results[c]["z"], runs)
        for g in range(G):
            m = mask[c, g]
            borig = sel[c, g][m]
            ig = indices[g, borig]
            zz = (zM[g][m] + b2[g, ig]
                  + eps[g, borig] * np.exp(0.5 * zL[g][m] + 0.5 * b2[g, LAT + ig]))
            z[g, borig] = zz
    return z.astype(np.float32)


# revision 24
# speedup vs baseline: 1.8121x; 1.0059x over previous
"""EnVAE sampling kernel for 8x TRN2 NeuronCores.

Math (per group g, batch element b):
  Xg = X[:, g::8]                                      # (b, 128)
  h  = relu(Xg @ W1[g] + b1[g])                        # (b, 128)
  out= h @ W2[g] + b2[g]; means=out[:, :64]; lv=out[:, 64:]
  z  = means[b, i] + eps * exp(0.5 * lv[b, i]),  i = indices[g, b]

Strategy: the latent index i is known on the host, so per group we sort the
batch by i and pad each (group, latent, core) bucket to a uniform quota.
After sorting, i is piecewise-constant in runs, so the "compute all 64
means/logvars then select" step collapses into per-run matmuls with a [128,2]
stationary = the selected (W2m[:,i], W2v[:,i]) column pair, producing
(zM, zL) = (selected mean-part, selected logvar-part) directly.  The host
finishes: z = zM + b2m[i] + eps * exp(0.5*zL + 0.5*b2v[i]).

mm1 runs in fp8(e4m3) DoubleRow perf mode (contraction 128 = 2 k-tiles of
64), mm2 in fp16.  No onehot tensors, no device-side exp/Hadamard - the only
elementwise work on device is the relu and the psum->sbuf output copies.
"""

import hashlib
import numpy as np
import ml_dtypes

import concourse.bass as bass
import concourse.bacc as bacc
import concourse.mybir as mybir
from concourse import tile
from concourse import bass_utils

OBS = 1024
LAT = 64
G = 8
GS = 128
HID = 128
BATCH = 65536
NCORES = 8

BP = 8256                      # padded per-core batch (uniform bucket quotas)
CHUNKS = [512] * 16 + [64]     # per-core column chunks (psum tile widths)
NCHUNK = len(CHUNKS)           # 17
QW = 2048                      # X piece width (4 pieces cover 8192 cols)
NPIECE = 4
NINST = NCHUNK * G             # 136
NROUND = (NINST + 3) // 4      # 34 zsel psum rounds (4 slots / 1-bank tile)
NDGRP = (NROUND + 3) // 4      # 9 staging drain groups (4 rounds each)

FP8 = mybir.dt.float8e4
F16 = mybir.dt.float16
F32 = mybir.dt.float32
E4 = ml_dtypes.float8_e4m3

GROUP_IDX = np.stack([np.arange(n, OBS, G) for n in range(G)])  # (g, gs)

CHUNK_OFF = np.concatenate([[0], np.cumsum(CHUNKS)])


def emission_order():
    """(chunk, g) emission order: chunk-pairs of the same group share one
    2-bank hp psum tile and a single relu; tail chunks come last."""
    order = []
    for pair in range(8):
        for g in range(G):
            order.append((2 * pair, g))
            order.append((2 * pair + 1, g))
    for g in range(G):
        order.append((16, g))
    return order


# ------------------------------------------------------------------- plan --

def _plan(indices):
    """Uniform per-core bucket quotas + per-core element selection.

    Returns:
      quota: (G, LAT) int - per-core count for each (group, latent) bucket
      sel:   (NCORES, G, BP) int32 - original batch index at each slot
      mask:  (NCORES, G, BP) bool - slot holds a real (non-dummy) element
      runs:  list over g of list over chunk of [(l, s, e), ...] segments
             (identical for every core by construction)
    """
    quota = np.zeros((G, LAT), np.int64)
    sel = np.zeros((NCORES, G, BP), np.int32)
    mask = np.zeros((NCORES, G, BP), bool)
    for g in range(G):
        idg = indices[g]
        order = np.argsort(idg, kind="stable")
        counts = np.bincount(idg, minlength=LAT).astype(np.int64)
        P = -(-counts // NCORES)          # ceil(n/8)
        deficit = BP - int(P.sum())
        assert deficit >= 0, f"BP={BP} too small: need {P.sum()}"
        P[:deficit] += 1
        quota[g] = P
        starts = np.concatenate([[0], np.cumsum(counts)])
        boundaries = np.concatenate([[0], np.cumsum(P)])
        for l in range(LAT):
            n, p = int(counts[l]), int(P[l])
            if n == 0:
                continue  # sel stays 0 / mask False; device output ignored
            bucket = order[starts[l]:starts[l] + n]
            j = np.arange(p)
            for c in range(NCORES):
                pos = c * p + j
                real = pos < n
                sl = slice(boundaries[l], boundaries[l] + p)
                sel[c, g, sl] = bucket[np.minimum(pos, n - 1)]
                mask[c, g, sl] = real
    # run segments per (g, chunk): bucket boundaries cut by chunk edges
    runs = []
    for g in range(G):
        bnd = np.cumsum(quota[g])         # bucket end positions (last = BP)
        gruns = []
        for c in range(NCHUNK):
            c0, c1 = int(CHUNK_OFF[c]), int(CHUNK_OFF[c + 1])
            segs = []
            lo = c0
            for l in range(LAT):
                hi = int(bnd[l])
                if hi <= lo:
                    continue
                if lo >= c1:
                    break
                e = min(hi, c1)
                segs.append((l, lo - c0, e - c0))
                lo = e
            gruns.append(segs)
        runs.append(gruns)
    return quota, sel, mask, runs


# ---------------------------------------------------------------- program --

BUILD_CFG = {"relu_split": False, "defer": 4, "copy_first": False,
             "zsel_bufs": 2, "hp_bufs": 3, "hs_bufs": 8, "stg_bufs": 6,
             "zout_q": ("act", "pool")}


def build_program(runs, num_devices=NCORES):
    nc = bacc.Bacc("TRN2", target_bir_lowering=False, debug=False,
                   num_devices=num_devices)

    xq = nc.dram_tensor("xq", [NPIECE, G, 64, 2, QW], FP8,
                        kind="ExternalInput").ap()
    xtail = nc.dram_tensor("xtail", [G, 64, 2, 64], FP8,
                           kind="ExternalInput").ap()
    w1 = nc.dram_tensor("w1", [G, 64, 2, HID], FP8, kind="ExternalInput").ap()
    w2 = nc.dram_tensor("w2", [G, HID, LAT, 2], F16, kind="ExternalInput").ap()
    b1 = nc.dram_tensor("b1", [G, HID], F32, kind="ExternalInput").ap()
    # z: (drain grp, psum row, round blk, col); slot k at rows 32k..32k+12:
    # run ri of the slot's (g,chunk) -> rows 32k+2ri (zM), 32k+2ri+1 (zL)
    zout = nc.dram_tensor("z", [NDGRP, 112, 4, 512], F16,
                          kind="ExternalOutput").ap()

    from contextlib import ExitStack
    with tile.TileContext(nc) as tc, ExitStack() as st:
        cp = st.enter_context(tc.tile_pool(name="const", bufs=1))
        # pre-warm the ACT function table (Relu) off the critical path:
        # without this the 1.3us LoadActFuncSet fires with the first relu
        warm = cp.tile([1, 2], F32, tag="warm")
        nc.vector.memset(warm[:], 0.0)
        nc.scalar.activation(warm[:], warm[:],
                             mybir.ActivationFunctionType.Relu,
                             bias=0.0, scale=1.0)
        w1_sb = cp.tile([64, G, 2, HID], FP8, tag="w1")
        nc.sync.dma_start(w1_sb[:], w1.rearrange("g p t m -> p g t m"))
        b1_sb = cp.tile([HID, G], F32, tag="b1")
        w2_sb = cp.tile([HID, G, LAT, 2], F16, tag="w2")
        xtl = cp.tile([64, G, 2, 64], FP8, tag="xtl")

        xpool = st.enter_context(tc.tile_pool(name="xg", bufs=33))
        hppool = st.enter_context(tc.tile_pool(name="hp",
                                               bufs=BUILD_CFG["hp_bufs"],
                                               space="PSUM"))
        hspool = st.enter_context(tc.tile_pool(name="hs",
                                               bufs=BUILD_CFG["hs_bufs"]))
        zpool = st.enter_context(tc.tile_pool(name="zp",
                                              bufs=BUILD_CFG["zsel_bufs"],
                                              space="PSUM"))
        spool = st.enter_context(tc.tile_pool(name="stg",
                                              bufs=BUILD_CFG["stg_bufs"]))

        # relu/copies run on ACT or DVE (GPSIMD cannot touch PSUM);
        # pick by accumulated load so the faster ACT takes a larger share.
        eng_load = {"act": 0.0, "dve": 0.0}

        def pick_engine(cost_act, cost_dve):
            if eng_load["act"] + cost_act <= eng_load["dve"] + cost_dve:
                eng_load["act"] += cost_act
                return "act"
            eng_load["dve"] += cost_dve
            return "dve"

        def _relu_one(e, dst, src, g):
            if e == "act":
                nc.scalar.activation(dst, src,
                                     mybir.ActivationFunctionType.Relu,
                                     bias=b1_sb[:, g:g + 1], scale=1.0)
            else:
                nc.vector.tensor_scalar(dst, src, b1_sb[:, g:g + 1], 0.0,
                                        mybir.AluOpType.add,
                                        mybir.AluOpType.max)

        def emit_relu(hs_t, hp_t, g, width):
            e = pick_engine(width * 0.833 + 143, width * 1.042 + 125)
            _relu_one(e, hs_t[:, :width], hp_t[:, :width], g)

        def emit_copy(dst, src):
            e = pick_engine(570, 658)
            if e == "act":
                nc.scalar.copy(dst, src)
            else:
                nc.vector.tensor_copy(dst, src)

        state = {"zt": None, "stg": None}
        pending_mm2 = []

        def emit_mm2(item):
            # item = (em, hs_t, base, chunk, g): zsel slot em -> round r =
            # em//4 (one [128,512] 1-bank tile = 4 partition-offset slots)
            em, hs_t, base, chunk, g = item
            rnd = em // 4
            off = 32 * (em % 4)
            if em % 4 == 0:
                state["zt"] = zpool.tile([128, 512], F32, name=f"zt{rnd}",
                                         tag="zsel")
            zt = state["zt"]
            segs = runs[g][chunk]
            width = CHUNK_OFF[chunk + 1] - CHUNK_OFF[chunk]
            nr = len(segs)
            assert nr <= 8, f"chunk spans {nr} runs"
            l0 = segs[0][0]
            assert segs[-1][0] == l0 + nr - 1, "runs not consecutive"
            nc.tensor.matmul(zt[off:off + 2 * nr, :width],
                             w2_sb[:, g, l0:l0 + nr],
                             hs_t[:, base:base + width],
                             start=True, stop=True,
                             skip_group_check=True,
                             tile_position=(0, off))
            if em % 4 == 3 or em == NINST - 1:
                grp, blk = rnd // 4, rnd % 4
                if blk == 0:
                    state["stg"] = spool.tile([128, 4, 512], F16,
                                              name=f"stg{grp}", tag="stg")
                stg = state["stg"]
                emit_copy(stg[0:112, blk, :], zt[0:112, :])
                if blk == 3 or rnd == NROUND - 1:
                    nb = blk + 1
                    QS = {"pool": nc.gpsimd, "act": nc.scalar,
                          "dve": nc.vector, "sp": nc.sync}
                    q0, q1 = BUILD_CFG["zout_q"]
                    half = 2 if nb > 1 else 1
                    QS[q0].dma_start(zout[grp, :, 0:half, :],
                                     stg[0:112, 0:half, :])
                    if nb > half:
                        QS[q1].dma_start(zout[grp, :, half:nb, :],
                                         stg[0:112, half:nb, :])

        em = 0
        order = emission_order()
        # X pieces: piece p covers chunks 4p..4p+3; set 0 loads upfront,
        # set p+1 streams in one DMA per pair while set p is consumed.
        xg = {}

        def load_piece(p, gg):
            t = xpool.tile([64, 2, QW], FP8, name=f"x{p}_{gg}", tag="xg")
            nc.sync.dma_start(t[:], xq[p, gg])
            xg[(p, gg)] = t

        load_piece(0, 0)
        nc.sync.dma_start(b1_sb[:], b1.rearrange("g k -> k g"))
        load_piece(0, 1)
        # w2 is first needed by mm2 of pair 0 (~8us in); xtail only at the end
        nc.sync.dma_start(w2_sb[:], w2.rearrange("g k l j -> k g l j"))
        for gg in range(2, G):
            load_piece(0, gg)
        nc.sync.dma_start(xtl[:], xtail.rearrange("g p t m -> p g t m"))
        # pairs: (chunk, chunk+1) of one group share an hp/hs pair-tile
        i = 0
        while i < len(order):
            chunk, g = order[i]
            if chunk < 16:
                piece = chunk // 4
                if chunk % 4 == 0 and piece + 1 < NPIECE:
                    load_piece(piece + 1, g)
                c0, c1 = chunk, chunk + 1
                so0 = int(CHUNK_OFF[c0] % QW)
                so1 = int(CHUNK_OFF[c1] % QW)
                hp = hppool.tile([128, 1024], F32, tag="hp")
                nc.tensor.matmul(hp[:, 0:512], w1_sb[:, g],
                                 xg[(piece, g)][:, :, so0:so0 + 512],
                                 start=True, stop=True,
                                 perf_mode=mybir.MatmulPerfMode.DoubleRow)
                nc.tensor.matmul(hp[:, 512:1024], w1_sb[:, g],
                                 xg[(piece, g)][:, :, so1:so1 + 512],
                                 start=True, stop=True,
                                 perf_mode=mybir.MatmulPerfMode.DoubleRow)
                if BUILD_CFG["copy_first"]:
                    while len(pending_mm2) > BUILD_CFG["defer"]:
                        emit_mm2(pending_mm2.pop(0))
                hs_t = hspool.tile([128, 1024], F16, tag="hs")
                emit_relu(hs_t, hp, g, 1024)
                if not BUILD_CFG["copy_first"]:
                    while len(pending_mm2) > BUILD_CFG["defer"]:
                        emit_mm2(pending_mm2.pop(0))
                pending_mm2.append((em, hs_t, 0, c0, g))
                pending_mm2.append((em + 1, hs_t, 512, c1, g))
                em += 2
                i += 2
            else:
                hp = hppool.tile([128, 1024], F32, tag="hp")
                nc.tensor.matmul(hp[:, 0:64], w1_sb[:, g], xtl[:, g],
                                 start=True, stop=True,
                                 perf_mode=mybir.MatmulPerfMode.DoubleRow)
                hs_t = hspool.tile([128, 1024], F16, tag="hs")
                emit_relu(hs_t, hp, g, 64)
                while len(pending_mm2) > min(BUILD_CFG["defer"], 2):
                    emit_mm2(pending_mm2.pop(0))
                pending_mm2.append((em, hs_t, 0, chunk, g))
                em += 1
                i += 1
        for item in pending_mm2:
            emit_mm2(item)

    nc.compile()
    return nc


# ------------------------------------------------------------------- host --

def _prep_inputs(X, W1, b1, W2, sel):
    """Per-core input dicts (xq/xtail/w1/w2/b1)."""
    w1_dev = np.ascontiguousarray(
        W1.astype(E4).reshape(G, 2, 64, HID).transpose(0, 2, 1, 3))
    # w2 packed (g, k, l, j): j=0 -> mean col l, j=1 -> logvar col l
    w2_dev = np.ascontiguousarray(
        W2.astype(np.float16).reshape(G, HID, 2, LAT).transpose(0, 1, 3, 2))
    b1_dev = b1.astype(np.float32)
    in_maps = []
    for c in range(NCORES):
        xq_c = np.empty((NPIECE, G, 64, 2, QW), E4)
        xtail_c = np.empty((G, 64, 2, 64), E4)
        for g in range(G):
            Xc = X[sel[c, g]][:, GROUP_IDX[g]].astype(E4)   # (BP, 128)
            Xt = np.ascontiguousarray(Xc.T)                  # (128, BP)
            blk = Xt.reshape(2, 64, BP)                      # (t, p, col)
            xq_c[:, g] = (blk[:, :, :NPIECE * QW]
                          .reshape(2, 64, NPIECE, QW).transpose(2, 1, 0, 3))
            xtail_c[g] = blk[:, :, NPIECE * QW:].transpose(1, 0, 2)
        in_maps.append({"xq": xq_c, "xtail": xtail_c, "w1": w1_dev,
                        "w2": w2_dev, "b1": b1_dev})
    return in_maps


def _decode(zres, runs):
    """(NDGRP,112,4,512) f16 device output -> zM, zL each (G, BP)."""
    zM = np.empty((G, BP), np.float32)
    zL = np.empty((G, BP), np.float32)
    for em, (chunk, g) in enumerate(emission_order()):
        rnd = em // 4
        grp, blk = rnd // 4, rnd % 4
        k = em % 4
        c0 = CHUNK_OFF[chunk]
        for ri, (l, s, e) in enumerate(runs[g][chunk]):
            zM[g, c0 + s:c0 + e] = zres[grp, 32 * k + 2 * ri, blk, s:e]
            zL[g, c0 + s:c0 + e] = zres[grp, 32 * k + 2 * ri + 1, blk, s:e]
    return zM, zL


_NC_CACHE = {}


def kernel(X, eps, W1, b1, W2, b2, indices):
    X = np.asarray(X, np.float32)
    eps = np.asarray(eps, np.float32)
    W1 = np.asarray(W1, np.float32)
    b1 = np.asarray(b1, np.float32)
    W2 = np.asarray(W2, np.float32)
    b2 = np.asarray(b2, np.float32)
    indices = np.asarray(indices, np.int32)

    key = hashlib.sha256(indices.tobytes()).hexdigest()
    if key not in _NC_CACHE:
        quota, sel, mask, runs = _plan(indices)
        nc = build_program(runs, NCORES)
        _NC_CACHE.clear()
        _NC_CACHE[key] = (nc, sel, mask, runs)
    nc, sel, mask, runs = _NC_CACHE[key]

    in_maps = _prep_inputs(X, W1, b1, W2, sel)
    res = bass_utils.run_bass_kernel_spmd(nc, in_maps,
                                          core_ids=list(range(NCORES)))

    z = np.zeros((G, BATCH), np.float32)
    for c in range(NCORES):
        zM, zL = _decode(res.results[c]["z"], runs)
        for g in range(G):
            m = mask[c, g]
            borig = sel[c, g][m]
            ig = indices[g, borig]
            zz = (zM[g][m] + b2[g, ig]
                  + eps[g, borig] * np.exp(0.5 * zL[g][m] + 0.5 * b2[g, LAT + ig]))
            z[g, borig] = zz
    return z.astype(np.float32)


# revision 25
# speedup vs baseline: 1.8253x; 1.0073x over previous
"""EnVAE sampling kernel for 8x TRN2 NeuronCores.

Math (per group g, batch element b):
  Xg = X[:, g::8]                                      # (b, 128)
  h  = relu(Xg @ W1[g] + b1[g])                        # (b, 128)
  out= h @ W2[g] + b2[g]; means=out[:, :64]; lv=out[:, 64:]
  z  = means[b, i] + eps * exp(0.5 * lv[b, i]),  i = indices[g, b]

Strategy: the latent index i is known on the host, so per group we sort the
batch by i and pad each (group, latent, core) bucket to a uniform quota.
After sorting, i is piecewise-constant in runs, so the "compute all 64
means/logvars then select" step collapses into per-run matmuls with a [128,2]
stationary = the selected (W2m[:,i], W2v[:,i]) column pair, producing
(zM, zL) = (selected mean-part, selected logvar-part) directly.  The host
finishes: z = zM + b2m[i] + eps * exp(0.5*zL + 0.5*b2v[i]).

mm1 runs in fp8(e4m3) DoubleRow perf mode (contraction 128 = 2 k-tiles of
64), mm2 in fp16.  No onehot tensors, no device-side exp/Hadamard - the only
elementwise work on device is the relu and the psum->sbuf output copies.
"""

import hashlib
import numpy as np
import ml_dtypes

import concourse.bass as bass
import concourse.bacc as bacc
import concourse.mybir as mybir
from concourse import tile
from concourse import bass_utils

OBS = 1024
LAT = 64
G = 8
GS = 128
HID = 128
BATCH = 65536
NCORES = 8

BP = 8256                      # padded per-core batch (uniform bucket quotas)
CHUNKS = [512] * 16 + [64]     # per-core column chunks (psum tile widths)
NCHUNK = len(CHUNKS)           # 17
QW = 2048                      # X piece width (4 pieces cover 8192 cols)
NPIECE = 4
NINST = NCHUNK * G             # 136
NROUND = (NINST + 3) // 4      # 34 zsel psum rounds (4 slots / 1-bank tile)
NDGRP = (NROUND + 3) // 4      # 9 staging drain groups (4 rounds each)

FP8 = mybir.dt.float8e4
F16 = mybir.dt.float16
F32 = mybir.dt.float32
E4 = ml_dtypes.float8_e4m3

GROUP_IDX = np.stack([np.arange(n, OBS, G) for n in range(G)])  # (g, gs)

CHUNK_OFF = np.concatenate([[0], np.cumsum(CHUNKS)])


def emission_order():
    """(chunk, g) emission order: the tiny tail chunks run FIRST (they fill
    the startup bubble while X pieces stream in); chunk-pairs of the same
    group then share one 2-bank hp psum tile and a single relu."""
    order = []
    for g in range(G):
        order.append((16, g))
    for pair in range(8):
        for g in range(G):
            order.append((2 * pair, g))
            order.append((2 * pair + 1, g))
    return order


# ------------------------------------------------------------------- plan --

def _plan(indices):
    """Uniform per-core bucket quotas + per-core element selection.

    Returns:
      quota: (G, LAT) int - per-core count for each (group, latent) bucket
      sel:   (NCORES, G, BP) int32 - original batch index at each slot
      mask:  (NCORES, G, BP) bool - slot holds a real (non-dummy) element
      runs:  list over g of list over chunk of [(l, s, e), ...] segments
             (identical for every core by construction)
    """
    quota = np.zeros((G, LAT), np.int64)
    sel = np.zeros((NCORES, G, BP), np.int32)
    mask = np.zeros((NCORES, G, BP), bool)
    for g in range(G):
        idg = indices[g]
        order = np.argsort(idg, kind="stable")
        counts = np.bincount(idg, minlength=LAT).astype(np.int64)
        P = -(-counts // NCORES)          # ceil(n/8)
        deficit = BP - int(P.sum())
        assert deficit >= 0, f"BP={BP} too small: need {P.sum()}"
        P[:deficit] += 1
        quota[g] = P
        starts = np.concatenate([[0], np.cumsum(counts)])
        boundaries = np.concatenate([[0], np.cumsum(P)])
        for l in range(LAT):
            n, p = int(counts[l]), int(P[l])
            if n == 0:
                continue  # sel stays 0 / mask False; device output ignored
            bucket = order[starts[l]:starts[l] + n]
            j = np.arange(p)
            for c in range(NCORES):
                pos = c * p + j
                real = pos < n
                sl = slice(boundaries[l], boundaries[l] + p)
                sel[c, g, sl] = bucket[np.minimum(pos, n - 1)]
                mask[c, g, sl] = real
    # run segments per (g, chunk): bucket boundaries cut by chunk edges
    runs = []
    for g in range(G):
        bnd = np.cumsum(quota[g])         # bucket end positions (last = BP)
        gruns = []
        for c in range(NCHUNK):
            c0, c1 = int(CHUNK_OFF[c]), int(CHUNK_OFF[c + 1])
            segs = []
            lo = c0
            for l in range(LAT):
                hi = int(bnd[l])
                if hi <= lo:
                    continue
                if lo >= c1:
                    break
                e = min(hi, c1)
                segs.append((l, lo - c0, e - c0))
                lo = e
            gruns.append(segs)
        runs.append(gruns)
    return quota, sel, mask, runs


# ---------------------------------------------------------------- program --

BUILD_CFG = {"relu_split": False, "defer": 4, "copy_first": False,
             "zsel_bufs": 2, "hp_bufs": 3, "hs_bufs": 8, "stg_bufs": 6,
             "zout_q": ("act", "pool")}


def build_program(runs, num_devices=NCORES):
    nc = bacc.Bacc("TRN2", target_bir_lowering=False, debug=False,
                   num_devices=num_devices)

    xq = nc.dram_tensor("xq", [NPIECE, G, 64, 2, QW], FP8,
                        kind="ExternalInput").ap()
    xtail = nc.dram_tensor("xtail", [G, 64, 2, 64], FP8,
                           kind="ExternalInput").ap()
    w1 = nc.dram_tensor("w1", [G, 64, 2, HID], FP8, kind="ExternalInput").ap()
    w2 = nc.dram_tensor("w2", [G, HID, LAT, 2], F16, kind="ExternalInput").ap()
    b1 = nc.dram_tensor("b1", [G, HID], F32, kind="ExternalInput").ap()
    # z: (drain grp, psum row, round blk, col); slot k at rows 32k..32k+12:
    # run ri of the slot's (g,chunk) -> rows 32k+2ri (zM), 32k+2ri+1 (zL)
    zout = nc.dram_tensor("z", [NDGRP, 112, 4, 512], F16,
                          kind="ExternalOutput").ap()

    from contextlib import ExitStack
    with tile.TileContext(nc) as tc, ExitStack() as st:
        cp = st.enter_context(tc.tile_pool(name="const", bufs=1))
        # pre-warm the ACT function table (Relu) off the critical path:
        # without this the 1.3us LoadActFuncSet fires with the first relu
        warm = cp.tile([1, 2], F32, tag="warm")
        nc.vector.memset(warm[:], 0.0)
        nc.scalar.activation(warm[:], warm[:],
                             mybir.ActivationFunctionType.Relu,
                             bias=0.0, scale=1.0)
        w1_sb = cp.tile([64, G, 2, HID], FP8, tag="w1")
        nc.sync.dma_start(w1_sb[:], w1.rearrange("g p t m -> p g t m"))
        b1_sb = cp.tile([HID, G], F32, tag="b1")
        w2_sb = cp.tile([HID, G, LAT, 2], F16, tag="w2")
        xtl = cp.tile([64, G, 2, 64], FP8, tag="xtl")

        xpool = st.enter_context(tc.tile_pool(name="xg", bufs=33))
        hppool = st.enter_context(tc.tile_pool(name="hp",
                                               bufs=BUILD_CFG["hp_bufs"],
                                               space="PSUM"))
        hspool = st.enter_context(tc.tile_pool(name="hs",
                                               bufs=BUILD_CFG["hs_bufs"]))
        zpool = st.enter_context(tc.tile_pool(name="zp",
                                              bufs=BUILD_CFG["zsel_bufs"],
                                              space="PSUM"))
        spool = st.enter_context(tc.tile_pool(name="stg",
                                              bufs=BUILD_CFG["stg_bufs"]))

        # relu/copies run on ACT or DVE (GPSIMD cannot touch PSUM);
        # pick by accumulated load so the faster ACT takes a larger share.
        eng_load = {"act": 0.0, "dve": 0.0}

        def pick_engine(cost_act, cost_dve):
            if eng_load["act"] + cost_act <= eng_load["dve"] + cost_dve:
                eng_load["act"] += cost_act
                return "act"
            eng_load["dve"] += cost_dve
            return "dve"

        def _relu_one(e, dst, src, g):
            if e == "act":
                nc.scalar.activation(dst, src,
                                     mybir.ActivationFunctionType.Relu,
                                     bias=b1_sb[:, g:g + 1], scale=1.0)
            else:
                nc.vector.tensor_scalar(dst, src, b1_sb[:, g:g + 1], 0.0,
                                        mybir.AluOpType.add,
                                        mybir.AluOpType.max)

        def emit_relu(hs_t, hp_t, g, width):
            e = pick_engine(width * 0.833 + 143, width * 1.042 + 125)
            _relu_one(e, hs_t[:, :width], hp_t[:, :width], g)

        def emit_copy(dst, src):
            e = pick_engine(570, 658)
            if e == "act":
                nc.scalar.copy(dst, src)
            else:
                nc.vector.tensor_copy(dst, src)

        state = {"zt": None, "stg": None}
        pending_mm2 = []

        def emit_mm2(item):
            # item = (em, hs_t, base, chunk, g): zsel slot em -> round r =
            # em//4 (one [128,512] 1-bank tile = 4 partition-offset slots)
            em, hs_t, base, chunk, g = item
            rnd = em // 4
            off = 32 * (em % 4)
            if em % 4 == 0:
                state["zt"] = zpool.tile([128, 512], F32, name=f"zt{rnd}",
                                         tag="zsel")
            zt = state["zt"]
            segs = runs[g][chunk]
            width = CHUNK_OFF[chunk + 1] - CHUNK_OFF[chunk]
            nr = len(segs)
            assert nr <= 8, f"chunk spans {nr} runs"
            l0 = segs[0][0]
            assert segs[-1][0] == l0 + nr - 1, "runs not consecutive"
            nc.tensor.matmul(zt[off:off + 2 * nr, :width],
                             w2_sb[:, g, l0:l0 + nr],
                             hs_t[:, base:base + width],
                             start=True, stop=True,
                             skip_group_check=True,
                             tile_position=(0, off))
            if em % 4 == 3 or em == NINST - 1:
                grp, blk = rnd // 4, rnd % 4
                if blk == 0:
                    state["stg"] = spool.tile([128, 4, 512], F16,
                                              name=f"stg{grp}", tag="stg")
                stg = state["stg"]
                emit_copy(stg[0:112, blk, :], zt[0:112, :])
                if blk == 3 or rnd == NROUND - 1:
                    nb = blk + 1
                    QS = {"pool": nc.gpsimd, "act": nc.scalar,
                          "dve": nc.vector, "sp": nc.sync}
                    q0, q1 = BUILD_CFG["zout_q"]
                    half = 2 if nb > 1 else 1
                    QS[q0].dma_start(zout[grp, :, 0:half, :],
                                     stg[0:112, 0:half, :])
                    if nb > half:
                        QS[q1].dma_start(zout[grp, :, half:nb, :],
                                         stg[0:112, half:nb, :])

        em = 0
        order = emission_order()
        # X pieces: piece p covers chunks 4p..4p+3; set 0 loads upfront,
        # set p+1 streams in one DMA per pair while set p is consumed.
        xg = {}

        def load_piece(p, gg):
            t = xpool.tile([64, 2, QW], FP8, name=f"x{p}_{gg}", tag="xg")
            nc.sync.dma_start(t[:], xq[p, gg])
            xg[(p, gg)] = t

        # tail chunks run first: xtl + b1 load before the X pieces
        nc.sync.dma_start(xtl[:], xtail.rearrange("g p t m -> p g t m"))
        nc.sync.dma_start(b1_sb[:], b1.rearrange("g k -> k g"))
        load_piece(0, 0)
        nc.sync.dma_start(w2_sb[:], w2.rearrange("g k l j -> k g l j"))
        for gg in range(1, G):
            load_piece(0, gg)
        # pairs: (chunk, chunk+1) of one group share an hp/hs pair-tile
        i = 0
        while i < len(order):
            chunk, g = order[i]
            if chunk < 16:
                piece = chunk // 4
                if chunk % 4 == 0 and piece + 1 < NPIECE:
                    load_piece(piece + 1, g)
                c0, c1 = chunk, chunk + 1
                so0 = int(CHUNK_OFF[c0] % QW)
                so1 = int(CHUNK_OFF[c1] % QW)
                hp = hppool.tile([128, 1024], F32, tag="hp")
                nc.tensor.matmul(hp[:, 0:512], w1_sb[:, g],
                                 xg[(piece, g)][:, :, so0:so0 + 512],
                                 start=True, stop=True,
                                 perf_mode=mybir.MatmulPerfMode.DoubleRow)
                nc.tensor.matmul(hp[:, 512:1024], w1_sb[:, g],
                                 xg[(piece, g)][:, :, so1:so1 + 512],
                                 start=True, stop=True,
                                 perf_mode=mybir.MatmulPerfMode.DoubleRow)
                if BUILD_CFG["copy_first"]:
                    while len(pending_mm2) > BUILD_CFG["defer"]:
                        emit_mm2(pending_mm2.pop(0))
                hs_t = hspool.tile([128, 1024], F16, tag="hs")
                emit_relu(hs_t, hp, g, 1024)
                if not BUILD_CFG["copy_first"]:
                    while len(pending_mm2) > BUILD_CFG["defer"]:
                        emit_mm2(pending_mm2.pop(0))
                pending_mm2.append((em, hs_t, 0, c0, g))
                pending_mm2.append((em + 1, hs_t, 512, c1, g))
                em += 2
                i += 2
            else:
                hp = hppool.tile([128, 1024], F32, tag="hp")
                nc.tensor.matmul(hp[:, 0:64], w1_sb[:, g], xtl[:, g],
                                 start=True, stop=True,
                                 perf_mode=mybir.MatmulPerfMode.DoubleRow)
                hs_t = hspool.tile([128, 1024], F16, tag="hs")
                emit_relu(hs_t, hp, g, 64)
                while len(pending_mm2) > min(BUILD_CFG["defer"], 2):
                    emit_mm2(pending_mm2.pop(0))
                pending_mm2.append((em, hs_t, 0, chunk, g))
                em += 1
                i += 1
        for item in pending_mm2:
            emit_mm2(item)

    nc.compile()
    return nc


# ------------------------------------------------------------------- host --

def _prep_inputs(X, W1, b1, W2, sel):
    """Per-core input dicts (xq/xtail/w1/w2/b1)."""
    w1_dev = np.ascontiguousarray(
        W1.astype(E4).reshape(G, 2, 64, HID).transpose(0, 2, 1, 3))
    # w2 packed (g, k, l, j): j=0 -> mean col l, j=1 -> logvar col l
    w2_dev = np.ascontiguousarray(
        W2.astype(np.float16).reshape(G, HID, 2, LAT).transpose(0, 1, 3, 2))
    b1_dev = b1.astype(np.float32)
    in_maps = []
    for c in range(NCORES):
        xq_c = np.empty((NPIECE, G, 64, 2, QW), E4)
        xtail_c = np.empty((G, 64, 2, 64), E4)
        for g in range(G):
            Xc = X[sel[c, g]][:, GROUP_IDX[g]].astype(E4)   # (BP, 128)
            Xt = np.ascontiguousarray(Xc.T)                  # (128, BP)
            blk = Xt.reshape(2, 64, BP)                      # (t, p, col)
            xq_c[:, g] = (blk[:, :, :NPIECE * QW]
                          .reshape(2, 64, NPIECE, QW).transpose(2, 1, 0, 3))
            xtail_c[g] = blk[:, :, NPIECE * QW:].transpose(1, 0, 2)
        in_maps.append({"xq": xq_c, "xtail": xtail_c, "w1": w1_dev,
                        "w2": w2_dev, "b1": b1_dev})
    return in_maps


def _decode(zres, runs):
    """(NDGRP,112,4,512) f16 device output -> zM, zL each (G, BP)."""
    zM = np.empty((G, BP), np.float32)
    zL = np.empty((G, BP), np.float32)
    for em, (chunk, g) in enumerate(emission_order()):
        rnd = em // 4
        grp, blk = rnd // 4, rnd % 4
        k = em % 4
        c0 = CHUNK_OFF[chunk]
        for ri, (l, s, e) in enumerate(runs[g][chunk]):
            zM[g, c0 + s:c0 + e] = zres[grp, 32 * k + 2 * ri, blk, s:e]
            zL[g, c0 + s:c0 + e] = zres[grp, 32 * k + 2 * ri + 1, blk, s:e]
    return zM, zL


_NC_CACHE = {}


def kernel(X, eps, W1, b1, W2, b2, indices):
    X = np.asarray(X, np.float32)
    eps = np.asarray(eps, np.float32)
    W1 = np.asarray(W1, np.float32)
    b1 = np.asarray(b1, np.float32)
    W2 = np.asarray(W2, np.float32)
    b2 = np.asarray(b2, np.float32)
    indices = np.asarray(indices, np.int32)

    key = hashlib.sha256(indices.tobytes()).hexdigest()
    if key not in _NC_CACHE:
        quota, sel, mask, runs = _plan(indices)
        nc = build_program(runs, NCORES)
        _NC_CACHE.clear()
        _NC_CACHE[key] = (nc, sel, mask, runs)
    nc, sel, mask, runs = _NC_CACHE[key]

    in_maps = _prep_inputs(X, W1, b1, W2, sel)
    res = bass_utils.run_bass_kernel_spmd(nc, in_maps,
                                          core_ids=list(range(NCORES)))

    z = np.zeros((G, BATCH), np.float32)
    for c in range(NCORES):
        zM, zL = _decode(res.results[c]["z"], runs)
        for g in range(G):
            m = mask[c, g]
            borig = sel[c, g][m]
            ig = indices[g, borig]
            zz = (zM[g][m] + b2[g, ig]
                  + eps[g, borig] * np.exp(0.5 * zL[g][m] + 0.5 * b2[g, LAT + ig]))
            z[g, borig] = zz
    return z.astype(np.float32)
